# revision 7
# baseline (speedup 1.0000x reference)
"""4-layer GCN encoder on 8 Trainium2 NeuronCores — ReduceScatter design.

Strategy (source-side partial aggregation):
  - Nodes are packed into 416 dst blocks of 128 by a two-phase balancer:
    cores by in-degree round-robin, then per-core greedy bin packing
    (squared-norm objective over per-source-core edge counts) so every
    (src core, dst block) bucket fits 2 edge tiles (1 for the last
    `t1blk` blocks of each core).  Core c owns blocks [c*52, (c+1)*52).
  - Edges are assigned to the core owning their SOURCE node.  Each
    layer, every core projects only its local shard (a = h @ W, kept in
    local DRAM) and gathers messages from that 3.4MB local table (int16
    row ids, no AllGather before aggregation).
  - Scatter-add on TensorE: pb[dst,f] += S^T M per 128-edge tile, where
    S = one-hot * norm.  S is built on DVE for half the batches and
    streamed from a layer-0 DRAM spill for the rest (smod), balancing
    DVE against the DMA engines.  PSUM evacuations are split DVE/Act
    (evmod).
  - Partial sums are staged to SBUF and written as bf16 partials; three
    chunked ReduceScatters (28/16/8 blocks per core) sum them across
    cores, each core receiving exactly its shard rows.  RS chunks are
    emitted a few buckets into the next segment (rsd) so their input
    waits don't stall the Pool gather stream; chunks A and B hide fully
    under the body, only the small C chunk is exposed.  RS-dependent
    epilogue DMAs go through SWDGE so they cannot alias-block HWDGE
    semaphore lanes used by body/prefetch DMAs.
  - Epilogue (rs + loopnorm*a_own + b, PReLU on the last layer) is
    batched per 4 blocks; the dense for the next layer runs inside it
    and lands in wide tiles flushed with one DMA per chunk.
"""

import numpy as np
import ml_dtypes

import concourse.bacc as bacc
import concourse.mybir as mybir
import concourse.tile as tile
from concourse.bass_utils import run_bass_kernel_spmd

P = 128
BF16 = mybir.dt.bfloat16
F32 = mybir.dt.float32
I16 = mybir.dt.int16


class Cfg:
    def __init__(self, n_nodes=50000, n_edges=800000, in_ch=512, hid=256,
                 ncores=8, nb=52, segs=(28, 16, 8), ct=8, gstage=8, rsd=3,
                 smod=2, evmod=3, t1blk=4):
        self.n_nodes = n_nodes
        self.n_edges = n_edges
        self.in_ch = in_ch
        self.hid = hid
        self.ncores = ncores
        self.nb = nb                      # dst blocks per core
        self.segs = tuple(segs)           # RS chunk sizes (blocks per core)
        assert sum(segs) == nb
        self.nbt = ncores * nb            # global dst blocks
        self.shard = nb * P               # nodes per core (padded)
        self.npad = self.nbt * P
        assert self.npad >= n_nodes
        assert self.shard < 32768         # int16 gather ids
        self.ct = ct                      # tiles per gather batch
        self.gstage = gstage              # dst blocks per staging flush
        self.rsd = rsd                    # B buckets emitted before RS_A
        self.smod = smod                  # 1/smod of S batches built on DVE
        self.evmod = evmod                # 1/evmod of evacs on Act (0=none)
        self.t1blk = t1blk                # single-tile blocks per core
        for sg in segs:
            assert (ncores * sg) % gstage == 0 and sg % 4 == 0
        self.fc_in = in_ch // P
        self.fh = hid // P


CFG = Cfg()


class Buckets:
    """Uniform (across cores) bucket schedule, in processing order."""

    def __init__(self, tb_ord, nsegs):
        self.tb = list(tb_ord)            # tiles per bucket, processing order
        self.nsegs = list(nsegs)          # buckets per RS segment
        self.off = np.zeros(len(self.tb) + 1, np.int64)
        self.off[1:] = np.cumsum(self.tb)
        self.ns = int(self.off[-1])

    def __repr__(self):
        return f"Buckets(n={len(self.tb)}, NS={self.ns}, segs={self.nsegs})"


# ----------------------------------------------------------------- host prep

def _preprocess(cfg, edge_index, edge_weight):
    N = cfg.n_nodes
    src = np.asarray(edge_index[0], dtype=np.int64)
    dst = np.asarray(edge_index[1], dtype=np.int64)
    ew = np.asarray(edge_weight, dtype=np.float32)
    deg = np.bincount(dst, weights=ew.astype(np.float64), minlength=N)
    deg = deg.astype(np.float32) + 1.0
    dis = (1.0 / np.sqrt(deg)).astype(np.float32)
    norm = dis[src] * ew * dis[dst]
    loopnorm = dis * dis

    # two-phase balanced assignment: cores by in-degree round-robin, then
    # per-core greedy (squared-norm objective) bin packing of nodes into
    # blocks so that per-(src core, dst block) edge counts stay <= 2 tiles
    degc = np.bincount(dst, minlength=N)
    order = np.argsort(-degc, kind="stable")
    core_of = np.empty(N, np.int64)
    core_of[order] = np.arange(N) % cfg.ncores
    kmat = np.zeros((N, cfg.ncores), np.int32)
    np.add.at(kmat, (dst, core_of[src]), 1)
    gslot = np.empty(N, dtype=np.int64)
    caps = np.full(cfg.nb, 2 * P, np.float64)
    caps[-cfg.t1blk:] = P            # small blocks: single-tile buckets
    for c in range(cfg.ncores):
        nodes = order[core_of[order] == c]
        loads = np.zeros((cfg.nb, cfg.ncores), np.float64)
        fill = np.zeros(cfg.nb, np.int64)
        for v in nodes:
            kv = kmat[v].astype(np.float64)
            score = loads @ kv
            over = (loads + kv[None, :]).max(axis=1) > caps
            score[over] += 1e12
            score[fill >= P] = 1e18
            b = int(np.argmin(score))
            loads[b] += kv
            gslot[v] = (c * cfg.nb + b) * P + fill[b]
            fill[b] += 1

    ps = gslot[src]
    pd = gslot[dst]
    core_e = ps // cfg.shard              # edge -> owning core (by src)
    gb = pd // P                          # global dst block

    # processing order of buckets: segment-major, core-major within segment
    c_dst = gb // cfg.nb
    bl = gb % cfg.nb
    seg_of = np.zeros(cfg.nb, np.int64)
    seg_start = np.zeros(cfg.nb, np.int64)
    seg_base = np.zeros(len(cfg.segs), np.int64)
    s0 = 0
    base = 0
    for si, sg in enumerate(cfg.segs):
        seg_of[s0:s0 + sg] = si
        seg_start[s0:s0 + sg] = s0
        seg_base[si] = base
        base += cfg.ncores * sg
        s0 += sg
    si_e = seg_of[bl]
    seglen_e = np.array(cfg.segs)[si_e]
    ordblk_e = (seg_base[si_e] + c_dst * seglen_e + (bl - seg_start[bl]))

    cnt = np.zeros((cfg.ncores, cfg.nbt), np.int64)
    np.add.at(cnt, (core_e, ordblk_e), 1)
    tb_ord = np.maximum((cnt.max(axis=0) + P - 1) // P, 1)
    bk = Buckets(tb_ord, [cfg.ncores * sg for sg in cfg.segs])
    nsp = ((bk.ns + cfg.ct - 1) // cfg.ct) * cfg.ct

    srt = np.lexsort((ps, ordblk_e, core_e))
    cs, os_ = core_e[srt], ordblk_e[srt]
    key = cs * cfg.nbt + os_
    uniq, inv, counts = np.unique(key, return_inverse=True, return_counts=True)
    starts = np.zeros_like(counts)
    starts[1:] = np.cumsum(counts)[:-1]
    rank = np.arange(len(srt)) - starts[inv]
    assert (rank < tb_ord[os_] * P).all()

    slot = bk.off[os_] + rank // P
    q = slot * P + rank % P
    gidx16 = np.zeros((cfg.ncores, 16, nsp * 8), dtype=np.int16)
    dstc = np.zeros((cfg.ncores, P, nsp), dtype=np.float32)
    normc = np.zeros((cfg.ncores, P, nsp), dtype=np.float32)
    sfull = np.zeros((cfg.ncores, P, nsp, P), dtype=ml_dtypes.bfloat16)
    idxval = (ps[srt] - cs * cfg.shard).astype(np.int16)
    dloci = (pd[srt] % P).astype(np.int64)
    dlocal = dloci.astype(np.float32)
    nval = norm[srt]
    for c in range(cfg.ncores):
        m = cs == c
        qc = q[m]
        gidx16[c, qc % 16, qc // 16] = idxval[m]
        dstc[c, qc % P, qc // P] = dlocal[m]
        normc[c, qc % P, qc // P] = nval[m]
        # host-built one-hot*norm scatter matrices (S), streamed on device
        sfull[c, qc % P, qc // P, dloci[m]] = nval[m]
    gidx = np.tile(gidx16, (1, 8, 1))

    lpad = np.zeros(cfg.npad, np.float32)
    lpad[gslot] = loopnorm
    loopn = lpad.reshape(cfg.ncores, cfg.nb, P).transpose(0, 2, 1)
    return dict(T=bk, nsp=nsp, gidx=gidx,
                dstc=dstc.astype(ml_dtypes.bfloat16),
                normc=normc.astype(ml_dtypes.bfloat16),
                sfull=sfull.reshape(cfg.ncores, P, nsp * P),
                loopn=np.ascontiguousarray(loopn),
                gslot=gslot)


def _pack_xts(cfg, x, gslot):
    xpad = np.zeros((cfg.npad, cfg.in_ch), dtype=np.float32)
    xpad[gslot] = x
    a = xpad.reshape(cfg.ncores, cfg.nb, P, cfg.fc_in, P)
    a = a.transpose(0, 1, 4, 3, 2).reshape(cfg.ncores, cfg.nb * P, cfg.fc_in * P)
    return np.ascontiguousarray(a.astype(ml_dtypes.bfloat16))


def _pack_wcat(cfg, Ws):
    cols = []
    for Wl in Ws:
        k = Wl.shape[0]
        for fc in range(k // P):
            cols.append(Wl[fc * P:(fc + 1) * P, :])
    return np.concatenate(cols, axis=1).astype(ml_dtypes.bfloat16)


def _iota_np():
    return np.tile(np.arange(P, dtype=np.float32)[None, :], (P, 1)).astype(
        ml_dtypes.bfloat16)


# ----------------------------------------------------------------- builder

def _build(cfg, bk, n_layers=4):
    nsp = ((bk.ns + cfg.ct - 1) // cfg.ct) * cfg.ct
    CT = cfg.ct
    HID = cfg.hid
    GS = cfg.gstage
    seg_rows = [n * P for n in bk.nsegs]
    nc = bacc.Bacc("TRN2", target_bir_lowering=False, debug=False,
                   num_devices=cfg.ncores, num_swdge_queues=4)
    qctr = [0]

    gidx_d = nc.dram_tensor("gidx", [P, nsp * 8], I16, kind="ExternalInput")
    sdump_d = nc.dram_tensor("sdump", [P, nsp * P], BF16, kind="ExternalInput")
    dstc_d = nc.dram_tensor("dstc", [P, nsp], BF16, kind="ExternalInput")
    normc_d = nc.dram_tensor("normc", [P, nsp], BF16, kind="ExternalInput")
    iota_d = nc.dram_tensor("iota", [P, P], BF16, kind="ExternalInput")
    ident_d = nc.dram_tensor("ident", [P, P], BF16, kind="ExternalInput")
    wcat_cols = (cfg.fc_in + (n_layers - 1) * cfg.fh) * HID
    wcat_d = nc.dram_tensor("wcat", [P, wcat_cols], BF16, kind="ExternalInput")
    brep_d = nc.dram_tensor("brep", [P, n_layers * HID], F32,
                            kind="ExternalInput")
    arep_d = nc.dram_tensor("arep", [P, HID], F32, kind="ExternalInput")
    loopn_d = nc.dram_tensor("loopn", [P, cfg.nb], F32, kind="ExternalInput")
    xts_d = nc.dram_tensor("xts", [cfg.nb * P, cfg.fc_in * P], BF16,
                           kind="ExternalInput")
    out_d = nc.dram_tensor("out", [cfg.nb * P, HID], F32,
                           kind="ExternalOutput")

    GE_EPI = 4
    w_off = {}
    off = 0
    for l in range(n_layers):
        k = cfg.fc_in if l == 0 else cfg.fh
        for fc in range(k):
            w_off[(l, fc)] = off
            off += HID

    with tile.TileContext(nc) as tc:
        with (
            tc.tile_pool(name="res", bufs=1) as res,
            tc.tile_pool(name="mpool", bufs=10) as mpool,
            tc.tile_pool(name="spool", bufs=10) as spool,
            tc.tile_pool(name="xpool", bufs=6) as xpool,
            tc.tile_pool(name="apool", bufs=1) as apool,
            tc.tile_pool(name="lbpool", bufs=1) as lbpool,
            tc.tile_pool(name="hpool", bufs=2) as hpool,
            tc.tile_pool(name="htpool", bufs=1) as htpool,
            tc.tile_pool(name="stgpool", bufs=2) as stgpool,
            tc.tile_pool(name="opool", bufs=2) as opool,
            tc.tile_pool(name="ppool", bufs=4, space="PSUM") as ppool,
            tc.tile_pool(name="tpool", bufs=2, space="PSUM") as tpool,
            tc.tile_pool(name="dpool", bufs=2, space="PSUM") as dpool,
            tc.tile_pool(name="dram", bufs=2, space="DRAM") as dram,
            tc.tile_pool(name="drp", bufs=2, space="DRAM") as drp,
            tc.tile_pool(name="drs", bufs=2, space="DRAM") as drs,
        ):
            # ---- resident loads
            gidx = res.tile([P, nsp * 8], I16)
            nc.sync.dma_start(out=gidx[:], in_=gidx_d[:])
            dstc = res.tile([P, nsp], BF16)
            nc.sync.dma_start(out=dstc[:], in_=dstc_d[:])
            normc = res.tile([P, nsp], BF16)
            nc.sync.dma_start(out=normc[:], in_=normc_d[:])
            iota = res.tile([P, P], BF16)
            nc.sync.dma_start(out=iota[:], in_=iota_d[:])
            ident = res.tile([P, P], BF16)
            nc.sync.dma_start(out=ident[:], in_=ident_d[:])
            wcat = res.tile([P, wcat_cols], BF16)
            nc.sync.dma_start(out=wcat[:], in_=wcat_d[:])
            brep = res.tile([P, n_layers * HID], F32)
            nc.sync.dma_start(out=brep[:], in_=brep_d[:])
            arep = res.tile([P, HID], F32)
            nc.sync.dma_start(out=arep[:], in_=arep_d[:])
            loopn = res.tile([P, cfg.nb], F32)
            nc.sync.dma_start(out=loopn[:], in_=loopn_d[:])

            aown = {}
            lbias = {}
            hT = {}
            owide = [None]

            def dense_block(l, nt, alocal_t):
                pd_ = dpool.tile([P, HID], F32, tag="pd", name="pd")
                if l == 0:
                    xsl = xpool.tile([P, cfg.fc_in * P], BF16, tag="xsl",
                                     name="xsl")
                    nc.sync.dma_start(out=xsl[:],
                                      in_=xts_d[nt * P:(nt + 1) * P, :])
                    nk = cfg.fc_in
                    for fc in range(nk):
                        nc.tensor.matmul(
                            out=pd_[:],
                            lhsT=xsl[:, fc * P:(fc + 1) * P],
                            rhs=wcat[:, w_off[(0, fc)]:w_off[(0, fc)] + HID],
                            start=(fc == 0), stop=(fc == nk - 1))
                else:
                    for fc in range(cfg.fh):
                        nc.tensor.matmul(
                            out=pd_[:],
                            lhsT=hT[nt][:, fc * P:(fc + 1) * P],
                            rhs=wcat[:, w_off[(l, fc)]:w_off[(l, fc)] + HID],
                            start=(fc == 0), stop=(fc == cfg.fh - 1))
                # dense outputs land in per-chunk wide tiles: one batched
                # alocal DMA per GE blocks instead of 56 small SWDGE ops
                ch, j = nt // GE_EPI, nt % GE_EPI
                if j == 0:
                    aown[ch] = apool.tile([P, GE_EPI * HID], BF16,
                                          tag=f"aw{ch}", name=f"aw{ch}")
                asb = aown[ch][:, j * HID:(j + 1) * HID]
                nc.scalar.copy(out=asb, in_=pd_[:])
                if j == GE_EPI - 1 or nt == cfg.nb - 1:
                    n0 = nt - j
                    nc.gpsimd.dma_start(
                        out=alocal_t[n0 * P:(nt + 1) * P, :].rearrange(
                            "(g p) f -> p g f", p=P),
                        in_=aown[ch][:, :(j + 1) * HID].rearrange(
                            "p (g f) -> p g f", f=HID))
                # self-loop + bias term, off the post-RS critical path:
                # lb = loopnorm * a_own + b
                lt = hpool.tile([P, HID], F32, tag="lt", name="lt")
                nc.vector.tensor_scalar(
                    out=lt[:], in0=asb,
                    scalar1=loopn[:, nt:nt + 1], scalar2=None,
                    op0=mybir.AluOpType.mult)
                lb = lbpool.tile([P, HID], BF16, tag=f"lb{nt}",
                                 name=f"lb{nt}")
                nc.vector.tensor_tensor(
                    out=lb[:], in0=lt[:], in1=brep[:, l * HID:(l + 1) * HID],
                    op=mybir.AluOpType.add)
                lbias[nt] = lb

            def body(l, alocal_t, p_ts):
                """Aggregate all buckets; write partials; fire RS_A/RS_B.
                Returns (rsA_t, rsB_t)."""
                batches = {}

                def get_batch(bi):
                    if bi in batches:
                        return batches[bi]
                    k0 = bi * CT
                    M = mpool.tile([P, CT * HID], BF16, tag="M", name="M")
                    nc.gpsimd.dma_gather(
                        out_ap=M[:].rearrange("p (t e) -> p t e", e=HID),
                        in_ap=alocal_t[:],
                        idxs_ap=gidx[:, k0 * 8:(k0 + CT) * 8],
                        num_idxs=CT * P,
                        num_idxs_reg=CT * P,
                        elem_size=HID,
                        queue_num=qctr[0] % 3,
                    )
                    qctr[0] += 1
                    S = spool.tile([P, CT * P], BF16, tag="S", name="S")
                    build = cfg.smod > 0 and bi % cfg.smod == cfg.smod - 1
                    if build:
                        # rebuild on DVE to relieve the DMA engines
                        s3 = S[:].rearrange("p (t e) -> p t e", e=P)
                        iob = iota[:].rearrange(
                            "p (o e) -> p o e", o=1).broadcast_to([P, CT, P])
                        nc.vector.tensor_tensor(
                            out=s3, in0=iob,
                            in1=dstc[:, k0:k0 + CT].to_broadcast([P, CT, P]),
                            op=mybir.AluOpType.is_equal)
                        nc.vector.tensor_tensor(
                            out=s3, in0=s3,
                            in1=normc[:, k0:k0 + CT].to_broadcast([P, CT, P]),
                            op=mybir.AluOpType.mult)
                    else:
                        nc.sync.dma_start(
                            out=S[:], in_=sdump_d[:, k0 * P:(k0 + CT) * P])
                    batches[bi] = (M, S)
                    return batches[bi]

                def emit_rs(seg, p_t, rows):
                    rs_t = drs.tile([rows // cfg.ncores, HID], BF16,
                                    tag=f"rs{seg}", name=f"rs{seg}")
                    nc.gpsimd.collective_compute(
                        "ReduceScatter",
                        mybir.AluOpType.add,
                        ins=[p_t[:].opt()],
                        outs=[rs_t[:].opt()],
                        replica_groups=[list(range(cfg.ncores))],
                    )
                    return rs_t

                nsg = len(bk.nsegs)
                rs = [None] * nsg
                stg = None
                base = 0
                for seg in range(nsg):
                    nseg = bk.nsegs[seg]
                    p_t = p_ts[seg]
                    for i in range(nseg):
                        if seg > 0 and i == cfg.rsd:
                            # emit the previous segment's RS only after a few
                            # gather batches of this segment are queued on
                            # Pool, so its input wait doesn't stall the
                            # gather stream
                            rs[seg - 1] = emit_rs(seg - 1, p_ts[seg - 1],
                                                  seg_rows[seg - 1])
                        bseq = base + i
                        off = int(bk.off[bseq])
                        tbn = int(bk.tb[bseq])
                        pb = ppool.tile([P, HID], F32, tag="pb", name="pb")
                        for t in range(tbn):
                            s = off + t
                            M, S = get_batch(s // CT)
                            j = s % CT
                            nc.tensor.matmul(
                                out=pb[:],
                                lhsT=S[:, j * P:(j + 1) * P],
                                rhs=M[:, j * HID:(j + 1) * HID],
                                start=(t == 0), stop=(t == tbn - 1))
                        gpos = i % GS
                        if gpos == 0:
                            stg = stgpool.tile([P, GS * HID], BF16, tag="stg",
                                               name="stg")
                        # evac PSUM->SBUF: DVE when idle (l>0), Act on l0
                        # (DVE builds S there); evmod shifts some to Act
                        on_act = (cfg.evmod > 0 and i % cfg.evmod == 1)
                        if not on_act:
                            nc.vector.tensor_scalar(
                                out=stg[:, gpos * HID:(gpos + 1) * HID],
                                in0=pb[:], scalar1=0.0, scalar2=None,
                                op0=mybir.AluOpType.add)
                        else:
                            nc.scalar.copy(
                                out=stg[:, gpos * HID:(gpos + 1) * HID],
                                in_=pb[:])
                        if gpos == GS - 1:
                            g0 = i - gpos
                            view = p_t[g0 * P:(g0 + GS) * P, :].rearrange(
                                "(g p) f -> p g f", p=P)
                            nc.sync.dma_start(
                                out=view,
                                in_=stg[:].rearrange("p (g f) -> p g f",
                                                     f=HID))
                    base += nseg
                    if seg == nsg - 1:
                        if rs[seg - 1] is None:
                            rs[seg - 1] = emit_rs(seg - 1, p_ts[seg - 1],
                                                  seg_rows[seg - 1])
                        rs[seg] = emit_rs(seg, p_t, seg_rows[seg])
                return rs

            def epilogue_block(l, nt, rsr, alocal_next):
                if l < n_layers - 1:
                    hsb = hpool.tile([P, HID], BF16, tag="hsb", name="hsb")
                    nc.vector.tensor_tensor(
                        out=hsb[:], in0=rsr, in1=lbias[nt][:],
                        op=mybir.AluOpType.add)
                    tp = tpool.tile([P, 2 * P], BF16, tag="tp", name="tp")
                    for fh in range(cfg.fh):
                        nc.tensor.transpose(
                            tp[:, fh * P:(fh + 1) * P],
                            hsb[:, fh * P:(fh + 1) * P], ident[:])
                    ht = htpool.tile([P, 2 * P], BF16, tag=f"hT{nt}",
                                     name=f"hT{nt}")
                    nc.scalar.copy(out=ht[:], in_=tp[:])
                    hT[nt] = ht
                    dense_block(l + 1, nt, alocal_next)
                else:
                    ve = nc.gpsimd if nt % 3 == 2 else nc.vector
                    hb2 = opool.tile([P, HID], F32, tag="hb2", name="hb2")
                    ve.tensor_tensor(
                        out=hb2[:], in0=rsr, in1=lbias[nt][:],
                        op=mybir.AluOpType.add)
                    t1 = opool.tile([P, HID], F32, tag="t1", name="t1")
                    ve.tensor_scalar(
                        out=t1[:], in0=hb2[:], scalar1=0.0, scalar2=None,
                        op0=mybir.AluOpType.min)
                    ve.tensor_tensor(
                        out=t1[:], in0=t1[:], in1=arep[:],
                        op=mybir.AluOpType.mult)
                    ch, j = nt // GE_EPI, nt % GE_EPI
                    if j == 0:
                        owide[0] = opool.tile([P, GE_EPI * HID], BF16,
                                              tag="ow", name="ow")
                    osl = owide[0][:, j * HID:(j + 1) * HID]
                    ve.tensor_scalar(
                        out=osl, in0=hb2[:], scalar1=0.0, scalar2=None,
                        op0=mybir.AluOpType.max)
                    ve.tensor_tensor(
                        out=osl, in0=osl, in1=t1[:],
                        op=mybir.AluOpType.add)
                    if j == GE_EPI - 1 or nt == cfg.nb - 1:
                        n0 = nt - j
                        # SWDGE cast bf16->f32 on the way out
                        nc.gpsimd.dma_start(
                            out=out_d[n0 * P:(nt + 1) * P, :].rearrange(
                                "(g p) f -> p g f", p=P),
                            in_=owide[0][:, :(j + 1) * HID].rearrange(
                                "p (g f) -> p g f", f=HID))

            # ---- layer pipeline
            alocal = dram.tile([cfg.shard, HID], BF16, tag="alocal",
                               name="alocal")
            for nt in range(cfg.nb):
                dense_block(0, nt, alocal)
            seg_lo = []
            s0 = 0
            for sg in cfg.segs:
                seg_lo.append(s0)
                s0 += sg
            for l in range(n_layers):
                p_ts = [drp.tile([r, HID], BF16, tag=f"p{si}", name=f"p{si}")
                        for si, r in enumerate(seg_rows)]
                rs_ts = body(l, alocal, p_ts)
                if l < n_layers - 1:
                    alocal = dram.tile([cfg.shard, HID], BF16, tag="alocal",
                                       name="alocal")
                # epilogues in chunks of GE blocks: one batched rs load each
                GE = GE_EPI
                for nt0 in range(0, cfg.nb, GE):
                    ng = min(GE, cfg.nb - nt0)
                    si = max(i for i in range(len(seg_lo))
                             if seg_lo[i] <= nt0)
                    assert nt0 + ng <= seg_lo[si] + cfg.segs[si]
                    rs_t, row0 = rs_ts[si], (nt0 - seg_lo[si]) * P
                    rsc = hpool.tile([P, GE * HID], BF16, tag="rsc",
                                     name="rsc")
                    nc.gpsimd.dma_start(
                        out=rsc[:, :ng * HID].rearrange("p (g f) -> p g f",
                                                        f=HID),
                        in_=rs_t[row0:row0 + ng * P, :].rearrange(
                            "(g p) f -> p g f", p=P))
                    for j in range(ng):
                        epilogue_block(l, nt0 + j,
                                       rsc[:, (j) * HID:(j + 1) * HID],
                                       alocal)

    nc.compile()
    return nc


# ----------------------------------------------------------------- execution

def _make_in_maps(cfg, prep, x, Ws, bs, prelu_a):
    xts = _pack_xts(cfg, np.asarray(x, np.float32), prep["gslot"])
    wcat = _pack_wcat(cfg, Ws)
    brep = np.zeros((P, 4 * cfg.hid), np.float32)
    for l, b in enumerate(bs):
        brep[:, l * cfg.hid:(l + 1) * cfg.hid] = b[None, :]
    arep = np.tile(np.asarray(prelu_a, np.float32)[None, :], (P, 1))
    iota = _iota_np()
    ident = np.eye(P, dtype=ml_dtypes.bfloat16)
    maps = []
    for c in range(cfg.ncores):
        maps.append({
            "gidx": prep["gidx"][c],
            "sdump": prep["sfull"][c],
            "dstc": prep["dstc"][c],
            "normc": prep["normc"][c],
            "iota": iota,
            "ident": ident,
            "wcat": wcat,
            "brep": brep,
            "arep": arep,
            "loopn": prep["loopn"][c],
            "xts": xts[c],
        })
    return maps


def _assemble_out(cfg, results, gslot):
    yperm = np.concatenate([results[c]["out"] for c in range(cfg.ncores)],
                           axis=0)
    return np.ascontiguousarray(yperm[gslot]).astype(np.float32)


def run(cfg, x, edge_index, edge_weight, W1, b1, W2, b2, W3, b3, W4, b4,
        prelu_a, return_nc=False):
    prep = _preprocess(cfg, edge_index, edge_weight)
    nc = _build(cfg, prep["T"])
    in_maps = _make_in_maps(cfg, prep, x,
                            [np.asarray(W1, np.float32), np.asarray(W2, np.float32),
                             np.asarray(W3, np.float32), np.asarray(W4, np.float32)],
                            [np.asarray(b1, np.float32), np.asarray(b2, np.float32),
                             np.asarray(b3, np.float32), np.asarray(b4, np.float32)],
                            np.asarray(prelu_a, np.float32))
    res = run_bass_kernel_spmd(nc, in_maps, core_ids=list(range(cfg.ncores)))
    y = _assemble_out(cfg, res.results, prep["gslot"])
    if return_nc:
        return y, nc, in_maps
    return y


def kernel(x, edge_index, edge_weight, W1, b1, W2, b2, W3, b3, W4, b4, prelu_a):
    return run(CFG, x, edge_index, edge_weight,
               W1, b1, W2, b2, W3, b3, W4, b4, prelu_a)


# revision 8
# speedup vs baseline: 1.0071x; 1.0071x over previous
"""4-layer GCN encoder on 8 Trainium2 NeuronCores — ReduceScatter design.

Strategy (source-side partial aggregation):
  - Nodes are packed into 416 dst blocks of 128 by a two-phase balancer:
    cores by in-degree round-robin, then per-core greedy bin packing
    (squared-norm objective over per-source-core edge counts) so every
    (src core, dst block) bucket fits 2 edge tiles (1 for the last
    `t1blk` blocks of each core).  Core c owns blocks [c*52, (c+1)*52).
  - Edges are assigned to the core owning their SOURCE node.  Each
    layer, every core projects only its local shard (a = h @ W, kept in
    local DRAM) and gathers messages from that 3.4MB local table (int16
    row ids, no AllGather before aggregation).
  - Scatter-add on TensorE: pb[dst,f] += S^T M per 128-edge tile, where
    S = one-hot * norm.  S is precomputed on the host (graph-only data,
    passed as an input); on device half the batches are streamed from
    DRAM and half rebuilt on DVE (smod), balancing the DMA engines
    against DVE.  PSUM evacuations are split DVE/Act (evmod).
  - Partial sums are staged to SBUF and written as bf16 partials; three
    chunked ReduceScatters (28/16/8 blocks per core) sum them across
    cores, each core receiving exactly its shard rows.  RS chunks are
    emitted a few buckets into the next segment (rsd) so their input
    waits don't stall the Pool gather stream; chunks A and B hide fully
    under the body, only the small C chunk is exposed.  RS-dependent
    epilogue DMAs go through SWDGE so they cannot alias-block HWDGE
    semaphore lanes used by body/prefetch DMAs.
  - Epilogue (rs + loopnorm*a_own + b, PReLU on the last layer) is
    batched per 4 blocks; the dense for the next layer runs inside it
    and lands in wide tiles flushed with one DMA per chunk.
"""

import numpy as np
import ml_dtypes

import concourse.bacc as bacc
import concourse.mybir as mybir
import concourse.tile as tile
from concourse.bass_utils import run_bass_kernel_spmd

P = 128
BF16 = mybir.dt.bfloat16
F32 = mybir.dt.float32
I16 = mybir.dt.int16


class Cfg:
    def __init__(self, n_nodes=50000, n_edges=800000, in_ch=512, hid=256,
                 ncores=8, nb=52, segs=(28, 16, 8), ct=8, gstage=8, rsd=3,
                 smod=2, evmod=3, t1blk=4):
        self.n_nodes = n_nodes
        self.n_edges = n_edges
        self.in_ch = in_ch
        self.hid = hid
        self.ncores = ncores
        self.nb = nb                      # dst blocks per core
        self.segs = tuple(segs)           # RS chunk sizes (blocks per core)
        assert sum(segs) == nb
        self.nbt = ncores * nb            # global dst blocks
        self.shard = nb * P               # nodes per core (padded)
        self.npad = self.nbt * P
        assert self.npad >= n_nodes
        assert self.shard < 32768         # int16 gather ids
        self.ct = ct                      # tiles per gather batch
        self.gstage = gstage              # dst blocks per staging flush
        self.rsd = rsd                    # B buckets emitted before RS_A
        self.smod = smod                  # 1/smod of S batches built on DVE
        self.evmod = evmod                # 1/evmod of evacs on Act (0=none)
        self.t1blk = t1blk                # single-tile blocks per core
        for sg in segs:
            assert (ncores * sg) % gstage == 0 and sg % 4 == 0
        self.fc_in = in_ch // P
        self.fh = hid // P


CFG = Cfg()


class Buckets:
    """Uniform (across cores) bucket schedule, in processing order."""

    def __init__(self, tb_ord, nsegs):
        self.tb = list(tb_ord)            # tiles per bucket, processing order
        self.nsegs = list(nsegs)          # buckets per RS segment
        self.off = np.zeros(len(self.tb) + 1, np.int64)
        self.off[1:] = np.cumsum(self.tb)
        self.ns = int(self.off[-1])

    def __repr__(self):
        return f"Buckets(n={len(self.tb)}, NS={self.ns}, segs={self.nsegs})"


# ----------------------------------------------------------------- host prep

def _preprocess(cfg, edge_index, edge_weight):
    N = cfg.n_nodes
    src = np.asarray(edge_index[0], dtype=np.int64)
    dst = np.asarray(edge_index[1], dtype=np.int64)
    ew = np.asarray(edge_weight, dtype=np.float32)
    deg = np.bincount(dst, weights=ew.astype(np.float64), minlength=N)
    deg = deg.astype(np.float32) + 1.0
    dis = (1.0 / np.sqrt(deg)).astype(np.float32)
    norm = dis[src] * ew * dis[dst]
    loopnorm = dis * dis

    # two-phase balanced assignment: cores by in-degree round-robin, then
    # per-core greedy (squared-norm objective) bin packing of nodes into
    # blocks so that per-(src core, dst block) edge counts stay <= 2 tiles
    degc = np.bincount(dst, minlength=N)
    order = np.argsort(-degc, kind="stable")
    core_of = np.empty(N, np.int64)
    core_of[order] = np.arange(N) % cfg.ncores
    kmat = np.zeros((N, cfg.ncores), np.int32)
    np.add.at(kmat, (dst, core_of[src]), 1)
    gslot = np.empty(N, dtype=np.int64)
    caps = np.full(cfg.nb, 2 * P, np.float64)
    caps[-cfg.t1blk:] = P            # small blocks: single-tile buckets
    for c in range(cfg.ncores):
        nodes = order[core_of[order] == c]
        loads = np.zeros((cfg.nb, cfg.ncores), np.float64)
        fill = np.zeros(cfg.nb, np.int64)
        for v in nodes:
            kv = kmat[v].astype(np.float64)
            score = loads @ kv
            over = (loads + kv[None, :]).max(axis=1) > caps
            score[over] += 1e12
            score[fill >= P] = 1e18
            b = int(np.argmin(score))
            loads[b] += kv
            gslot[v] = (c * cfg.nb + b) * P + fill[b]
            fill[b] += 1

    ps = gslot[src]
    pd = gslot[dst]
    core_e = ps // cfg.shard              # edge -> owning core (by src)
    gb = pd // P                          # global dst block

    # processing order of buckets: segment-major, core-major within segment
    c_dst = gb // cfg.nb
    bl = gb % cfg.nb
    seg_of = np.zeros(cfg.nb, np.int64)
    seg_start = np.zeros(cfg.nb, np.int64)
    seg_base = np.zeros(len(cfg.segs), np.int64)
    s0 = 0
    base = 0
    for si, sg in enumerate(cfg.segs):
        seg_of[s0:s0 + sg] = si
        seg_start[s0:s0 + sg] = s0
        seg_base[si] = base
        base += cfg.ncores * sg
        s0 += sg
    si_e = seg_of[bl]
    seglen_e = np.array(cfg.segs)[si_e]
    ordblk_e = (seg_base[si_e] + c_dst * seglen_e + (bl - seg_start[bl]))

    cnt = np.zeros((cfg.ncores, cfg.nbt), np.int64)
    np.add.at(cnt, (core_e, ordblk_e), 1)
    tb_ord = np.maximum((cnt.max(axis=0) + P - 1) // P, 1)
    bk = Buckets(tb_ord, [cfg.ncores * sg for sg in cfg.segs])
    nsp = ((bk.ns + cfg.ct - 1) // cfg.ct) * cfg.ct

    srt = np.lexsort((ps, ordblk_e, core_e))
    cs, os_ = core_e[srt], ordblk_e[srt]
    key = cs * cfg.nbt + os_
    uniq, inv, counts = np.unique(key, return_inverse=True, return_counts=True)
    starts = np.zeros_like(counts)
    starts[1:] = np.cumsum(counts)[:-1]
    rank = np.arange(len(srt)) - starts[inv]
    assert (rank < tb_ord[os_] * P).all()

    slot = bk.off[os_] + rank // P
    q = slot * P + rank % P
    gidx16 = np.zeros((cfg.ncores, 16, nsp * 8), dtype=np.int16)
    dstc = np.zeros((cfg.ncores, P, nsp), dtype=np.float32)
    normc = np.zeros((cfg.ncores, P, nsp), dtype=np.float32)
    sfull = np.zeros((cfg.ncores, P, nsp, P), dtype=ml_dtypes.bfloat16)
    idxval = (ps[srt] - cs * cfg.shard).astype(np.int16)
    dloci = (pd[srt] % P).astype(np.int64)
    dlocal = dloci.astype(np.float32)
    nval = norm[srt]
    for c in range(cfg.ncores):
        m = cs == c
        qc = q[m]
        gidx16[c, qc % 16, qc // 16] = idxval[m]
        dstc[c, qc % P, qc // P] = dlocal[m]
        normc[c, qc % P, qc // P] = nval[m]
        # host-built one-hot*norm scatter matrices (S), streamed on device
        sfull[c, qc % P, qc // P, dloci[m]] = nval[m]
    gidx = np.tile(gidx16, (1, 8, 1))

    lpad = np.zeros(cfg.npad, np.float32)
    lpad[gslot] = loopnorm
    loopn = lpad.reshape(cfg.ncores, cfg.nb, P).transpose(0, 2, 1)
    return dict(T=bk, nsp=nsp, gidx=gidx,
                dstc=dstc.astype(ml_dtypes.bfloat16),
                normc=normc.astype(ml_dtypes.bfloat16),
                sfull=sfull.reshape(cfg.ncores, P, nsp * P),
                loopn=np.ascontiguousarray(loopn),
                gslot=gslot)


def _pack_xts(cfg, x, gslot):
    xpad = np.zeros((cfg.npad, cfg.in_ch), dtype=np.float32)
    xpad[gslot] = x
    a = xpad.reshape(cfg.ncores, cfg.nb, P, cfg.fc_in, P)
    a = a.transpose(0, 1, 4, 3, 2).reshape(cfg.ncores, cfg.nb * P, cfg.fc_in * P)
    return np.ascontiguousarray(a.astype(ml_dtypes.bfloat16))


def _pack_wcat(cfg, Ws):
    cols = []
    for Wl in Ws:
        k = Wl.shape[0]
        for fc in range(k // P):
            cols.append(Wl[fc * P:(fc + 1) * P, :])
    return np.concatenate(cols, axis=1).astype(ml_dtypes.bfloat16)


def _iota_np():
    return np.tile(np.arange(P, dtype=np.float32)[None, :], (P, 1)).astype(
        ml_dtypes.bfloat16)


# ----------------------------------------------------------------- builder

def _build(cfg, bk, n_layers=4):
    nsp = ((bk.ns + cfg.ct - 1) // cfg.ct) * cfg.ct
    CT = cfg.ct
    HID = cfg.hid
    GS = cfg.gstage
    seg_rows = [n * P for n in bk.nsegs]
    nc = bacc.Bacc("TRN2", target_bir_lowering=False, debug=False,
                   num_devices=cfg.ncores, num_swdge_queues=4)
    qctr = [0]

    gidx_d = nc.dram_tensor("gidx", [P, nsp * 8], I16, kind="ExternalInput")
    sdump_d = nc.dram_tensor("sdump", [P, nsp * P], BF16, kind="ExternalInput")
    dstc_d = nc.dram_tensor("dstc", [P, nsp], BF16, kind="ExternalInput")
    normc_d = nc.dram_tensor("normc", [P, nsp], BF16, kind="ExternalInput")
    iota_d = nc.dram_tensor("iota", [P, P], BF16, kind="ExternalInput")
    ident_d = nc.dram_tensor("ident", [P, P], BF16, kind="ExternalInput")
    wcat_cols = (cfg.fc_in + (n_layers - 1) * cfg.fh) * HID
    wcat_d = nc.dram_tensor("wcat", [P, wcat_cols], BF16, kind="ExternalInput")
    brep_d = nc.dram_tensor("brep", [P, n_layers * HID], F32,
                            kind="ExternalInput")
    arep_d = nc.dram_tensor("arep", [P, HID], F32, kind="ExternalInput")
    loopn_d = nc.dram_tensor("loopn", [P, cfg.nb], F32, kind="ExternalInput")
    xts_d = nc.dram_tensor("xts", [cfg.nb * P, cfg.fc_in * P], BF16,
                           kind="ExternalInput")
    out_d = nc.dram_tensor("out", [cfg.nb * P, HID], F32,
                           kind="ExternalOutput")

    GE_EPI = 4
    w_off = {}
    off = 0
    for l in range(n_layers):
        k = cfg.fc_in if l == 0 else cfg.fh
        for fc in range(k):
            w_off[(l, fc)] = off
            off += HID

    with tile.TileContext(nc) as tc:
        with (
            tc.tile_pool(name="res", bufs=1) as res,
            tc.tile_pool(name="mpool", bufs=10) as mpool,
            tc.tile_pool(name="spool", bufs=10) as spool,
            tc.tile_pool(name="xpool", bufs=6) as xpool,
            tc.tile_pool(name="apool", bufs=1) as apool,
            tc.tile_pool(name="lbpool", bufs=1) as lbpool,
            tc.tile_pool(name="hpool", bufs=2) as hpool,
            tc.tile_pool(name="htpool", bufs=1) as htpool,
            tc.tile_pool(name="stgpool", bufs=2) as stgpool,
            tc.tile_pool(name="opool", bufs=2) as opool,
            tc.tile_pool(name="ppool", bufs=4, space="PSUM") as ppool,
            tc.tile_pool(name="tpool", bufs=2, space="PSUM") as tpool,
            tc.tile_pool(name="dpool", bufs=2, space="PSUM") as dpool,
            tc.tile_pool(name="dram", bufs=2, space="DRAM") as dram,
            tc.tile_pool(name="drp", bufs=2, space="DRAM") as drp,
            tc.tile_pool(name="drs", bufs=2, space="DRAM") as drs,
        ):
            # ---- resident loads
            gidx = res.tile([P, nsp * 8], I16)
            nc.sync.dma_start(out=gidx[:], in_=gidx_d[:])
            dstc = res.tile([P, nsp], BF16)
            nc.sync.dma_start(out=dstc[:], in_=dstc_d[:])
            normc = res.tile([P, nsp], BF16)
            nc.sync.dma_start(out=normc[:], in_=normc_d[:])
            iota = res.tile([P, P], BF16)
            nc.sync.dma_start(out=iota[:], in_=iota_d[:])
            ident = res.tile([P, P], BF16)
            nc.sync.dma_start(out=ident[:], in_=ident_d[:])
            wcat = res.tile([P, wcat_cols], BF16)
            nc.sync.dma_start(out=wcat[:], in_=wcat_d[:])
            brep = res.tile([P, n_layers * HID], F32)
            nc.sync.dma_start(out=brep[:], in_=brep_d[:])
            arep = res.tile([P, HID], F32)
            nc.sync.dma_start(out=arep[:], in_=arep_d[:])
            loopn = res.tile([P, cfg.nb], F32)
            nc.sync.dma_start(out=loopn[:], in_=loopn_d[:])

            aown = {}
            lbias = {}
            hT = {}
            owide = [None]

            def dense_block(l, nt, alocal_t):
                pd_ = dpool.tile([P, HID], F32, tag="pd", name="pd")
                if l == 0:
                    xsl = xpool.tile([P, cfg.fc_in * P], BF16, tag="xsl",
                                     name="xsl")
                    nc.sync.dma_start(out=xsl[:],
                                      in_=xts_d[nt * P:(nt + 1) * P, :])
                    nk = cfg.fc_in
                    for fc in range(nk):
                        nc.tensor.matmul(
                            out=pd_[:],
                            lhsT=xsl[:, fc * P:(fc + 1) * P],
                            rhs=wcat[:, w_off[(0, fc)]:w_off[(0, fc)] + HID],
                            start=(fc == 0), stop=(fc == nk - 1))
                else:
                    for fc in range(cfg.fh):
                        nc.tensor.matmul(
                            out=pd_[:],
                            lhsT=hT[nt][:, fc * P:(fc + 1) * P],
                            rhs=wcat[:, w_off[(l, fc)]:w_off[(l, fc)] + HID],
                            start=(fc == 0), stop=(fc == cfg.fh - 1))
                # dense outputs land in per-chunk wide tiles: one batched
                # alocal DMA per GE blocks instead of 56 small SWDGE ops
                ch, j = nt // GE_EPI, nt % GE_EPI
                if j == 0:
                    aown[ch] = apool.tile([P, GE_EPI * HID], BF16,
                                          tag=f"aw{ch}", name=f"aw{ch}")
                asb = aown[ch][:, j * HID:(j + 1) * HID]
                nc.scalar.copy(out=asb, in_=pd_[:])
                if j == GE_EPI - 1 or nt == cfg.nb - 1:
                    n0 = nt - j
                    nc.gpsimd.dma_start(
                        out=alocal_t[n0 * P:(nt + 1) * P, :].rearrange(
                            "(g p) f -> p g f", p=P),
                        in_=aown[ch][:, :(j + 1) * HID].rearrange(
                            "p (g f) -> p g f", f=HID))
                # self-loop + bias term, off the post-RS critical path:
                # lb = loopnorm * a_own + b
                lt = hpool.tile([P, HID], F32, tag="lt", name="lt")
                nc.vector.tensor_scalar(
                    out=lt[:], in0=asb,
                    scalar1=loopn[:, nt:nt + 1], scalar2=None,
                    op0=mybir.AluOpType.mult)
                lb = lbpool.tile([P, HID], BF16, tag=f"lb{nt}",
                                 name=f"lb{nt}")
                nc.vector.tensor_tensor(
                    out=lb[:], in0=lt[:], in1=brep[:, l * HID:(l + 1) * HID],
                    op=mybir.AluOpType.add)
                lbias[nt] = lb

            def body(l, alocal_t, p_ts):
                """Aggregate all buckets; write partials; fire RS_A/RS_B.
                Returns (rsA_t, rsB_t)."""
                batches = {}

                def get_batch(bi):
                    if bi in batches:
                        return batches[bi]
                    k0 = bi * CT
                    M = mpool.tile([P, CT * HID], BF16, tag="M", name="M")
                    nc.gpsimd.dma_gather(
                        out_ap=M[:].rearrange("p (t e) -> p t e", e=HID),
                        in_ap=alocal_t[:],
                        idxs_ap=gidx[:, k0 * 8:(k0 + CT) * 8],
                        num_idxs=CT * P,
                        num_idxs_reg=CT * P,
                        elem_size=HID,
                        queue_num=qctr[0] % 3,
                    )
                    qctr[0] += 1
                    S = spool.tile([P, CT * P], BF16, tag="S", name="S")
                    build = cfg.smod > 0 and bi % cfg.smod == cfg.smod - 1
                    if build:
                        # rebuild on DVE to relieve the DMA engines
                        s3 = S[:].rearrange("p (t e) -> p t e", e=P)
                        iob = iota[:].rearrange(
                            "p (o e) -> p o e", o=1).broadcast_to([P, CT, P])
                        nc.vector.tensor_tensor(
                            out=s3, in0=iob,
                            in1=dstc[:, k0:k0 + CT].to_broadcast([P, CT, P]),
                            op=mybir.AluOpType.is_equal)
                        nc.vector.tensor_tensor(
                            out=s3, in0=s3,
                            in1=normc[:, k0:k0 + CT].to_broadcast([P, CT, P]),
                            op=mybir.AluOpType.mult)
                    else:
                        nc.sync.dma_start(
                            out=S[:], in_=sdump_d[:, k0 * P:(k0 + CT) * P])
                    batches[bi] = (M, S)
                    return batches[bi]

                def emit_rs(seg, p_t, rows):
                    rs_t = drs.tile([rows // cfg.ncores, HID], BF16,
                                    tag=f"rs{seg}", name=f"rs{seg}")
                    nc.gpsimd.collective_compute(
                        "ReduceScatter",
                        mybir.AluOpType.add,
                        ins=[p_t[:].opt()],
                        outs=[rs_t[:].opt()],
                        replica_groups=[list(range(cfg.ncores))],
                    )
                    return rs_t

                nsg = len(bk.nsegs)
                rs = [None] * nsg
                stg = None
                base = 0
                for seg in range(nsg):
                    nseg = bk.nsegs[seg]
                    p_t = p_ts[seg]
                    for i in range(nseg):
                        if seg > 0 and i == cfg.rsd:
                            # emit the previous segment's RS only after a few
                            # gather batches of this segment are queued on
                            # Pool, so its input wait doesn't stall the
                            # gather stream
                            rs[seg - 1] = emit_rs(seg - 1, p_ts[seg - 1],
                                                  seg_rows[seg - 1])
                        bseq = base + i
                        off = int(bk.off[bseq])
                        tbn = int(bk.tb[bseq])
                        pb = ppool.tile([P, HID], F32, tag="pb", name="pb")
                        for t in range(tbn):
                            s = off + t
                            M, S = get_batch(s // CT)
                            j = s % CT
                            nc.tensor.matmul(
                                out=pb[:],
                                lhsT=S[:, j * P:(j + 1) * P],
                                rhs=M[:, j * HID:(j + 1) * HID],
                                start=(t == 0), stop=(t == tbn - 1))
                        gpos = i % GS
                        if gpos == 0:
                            stg = stgpool.tile([P, GS * HID], BF16, tag="stg",
                                               name="stg")
                        # evac PSUM->SBUF: DVE when idle (l>0), Act on l0
                        # (DVE builds S there); evmod shifts some to Act
                        on_act = (cfg.evmod > 0 and i % cfg.evmod == 1)
                        if not on_act:
                            nc.vector.tensor_scalar(
                                out=stg[:, gpos * HID:(gpos + 1) * HID],
                                in0=pb[:], scalar1=0.0, scalar2=None,
                                op0=mybir.AluOpType.add)
                        else:
                            nc.scalar.copy(
                                out=stg[:, gpos * HID:(gpos + 1) * HID],
                                in_=pb[:])
                        if gpos == GS - 1:
                            g0 = i - gpos
                            view = p_t[g0 * P:(g0 + GS) * P, :].rearrange(
                                "(g p) f -> p g f", p=P)
                            nc.sync.dma_start(
                                out=view,
                                in_=stg[:].rearrange("p (g f) -> p g f",
                                                     f=HID))
                    base += nseg
                    if seg == nsg - 1:
                        if rs[seg - 1] is None:
                            rs[seg - 1] = emit_rs(seg - 1, p_ts[seg - 1],
                                                  seg_rows[seg - 1])
                        rs[seg] = emit_rs(seg, p_t, seg_rows[seg])
                return rs

            def epilogue_block(l, nt, rsr, alocal_next):
                if l < n_layers - 1:
                    hsb = hpool.tile([P, HID], BF16, tag="hsb", name="hsb")
                    nc.vector.tensor_tensor(
                        out=hsb[:], in0=rsr, in1=lbias[nt][:],
                        op=mybir.AluOpType.add)
                    tp = tpool.tile([P, 2 * P], BF16, tag="tp", name="tp")
                    for fh in range(cfg.fh):
                        nc.tensor.transpose(
                            tp[:, fh * P:(fh + 1) * P],
                            hsb[:, fh * P:(fh + 1) * P], ident[:])
                    ht = htpool.tile([P, 2 * P], BF16, tag=f"hT{nt}",
                                     name=f"hT{nt}")
                    nc.scalar.copy(out=ht[:], in_=tp[:])
                    hT[nt] = ht
                    dense_block(l + 1, nt, alocal_next)
                else:
                    ve = nc.gpsimd if nt % 3 == 2 else nc.vector
                    hb2 = opool.tile([P, HID], F32, tag="hb2", name="hb2")
                    ve.tensor_tensor(
                        out=hb2[:], in0=rsr, in1=lbias[nt][:],
                        op=mybir.AluOpType.add)
                    t1 = opool.tile([P, HID], F32, tag="t1", name="t1")
                    ve.tensor_scalar(
                        out=t1[:], in0=hb2[:], scalar1=0.0, scalar2=None,
                        op0=mybir.AluOpType.min)
                    ve.tensor_tensor(
                        out=t1[:], in0=t1[:], in1=arep[:],
                        op=mybir.AluOpType.mult)
                    ch, j = nt // GE_EPI, nt % GE_EPI
                    if j == 0:
                        owide[0] = opool.tile([P, GE_EPI * HID], BF16,
                                              tag="ow", name="ow")
                    osl = owide[0][:, j * HID:(j + 1) * HID]
                    ve.tensor_scalar(
                        out=osl, in0=hb2[:], scalar1=0.0, scalar2=None,
                        op0=mybir.AluOpType.max)
                    ve.tensor_tensor(
                        out=osl, in0=osl, in1=t1[:],
                        op=mybir.AluOpType.add)
                    if j == GE_EPI - 1 or nt == cfg.nb - 1:
                        n0 = nt - j
                        # SWDGE cast bf16->f32 on the way out
                        nc.gpsimd.dma_start(
                            out=out_d[n0 * P:(nt + 1) * P, :].rearrange(
                                "(g p) f -> p g f", p=P),
                            in_=owide[0][:, :(j + 1) * HID].rearrange(
                                "p (g f) -> p g f", f=HID))

            # ---- layer pipeline
            alocal = dram.tile([cfg.shard, HID], BF16, tag="alocal",
                               name="alocal")
            for nt in range(cfg.nb):
                dense_block(0, nt, alocal)
            seg_lo = []
            s0 = 0
            for sg in cfg.segs:
                seg_lo.append(s0)
                s0 += sg
            for l in range(n_layers):
                p_ts = [drp.tile([r, HID], BF16, tag=f"p{si}", name=f"p{si}")
                        for si, r in enumerate(seg_rows)]
                rs_ts = body(l, alocal, p_ts)
                if l < n_layers - 1:
                    alocal = dram.tile([cfg.shard, HID], BF16, tag="alocal",
                                       name="alocal")
                # epilogues in chunks of GE blocks: one batched rs load each
                GE = GE_EPI
                for nt0 in range(0, cfg.nb, GE):
                    ng = min(GE, cfg.nb - nt0)
                    si = max(i for i in range(len(seg_lo))
                             if seg_lo[i] <= nt0)
                    assert nt0 + ng <= seg_lo[si] + cfg.segs[si]
                    rs_t, row0 = rs_ts[si], (nt0 - seg_lo[si]) * P
                    rsc = hpool.tile([P, GE * HID], BF16, tag="rsc",
                                     name="rsc")
                    nc.gpsimd.dma_start(
                        out=rsc[:, :ng * HID].rearrange("p (g f) -> p g f",
                                                        f=HID),
                        in_=rs_t[row0:row0 + ng * P, :].rearrange(
                            "(g p) f -> p g f", p=P))
                    for j in range(ng):
                        epilogue_block(l, nt0 + j,
                                       rsc[:, (j) * HID:(j + 1) * HID],
                                       alocal)

    nc.compile()
    return nc


# ----------------------------------------------------------------- execution

def _make_in_maps(cfg, prep, x, Ws, bs, prelu_a):
    xts = _pack_xts(cfg, np.asarray(x, np.float32), prep["gslot"])
    wcat = _pack_wcat(cfg, Ws)
    brep = np.zeros((P, 4 * cfg.hid), np.float32)
    for l, b in enumerate(bs):
        brep[:, l * cfg.hid:(l + 1) * cfg.hid] = b[None, :]
    arep = np.tile(np.asarray(prelu_a, np.float32)[None, :], (P, 1))
    iota = _iota_np()
    ident = np.eye(P, dtype=ml_dtypes.bfloat16)
    maps = []
    for c in range(cfg.ncores):
        maps.append({
            "gidx": prep["gidx"][c],
            "sdump": prep["sfull"][c],
            "dstc": prep["dstc"][c],
            "normc": prep["normc"][c],
            "iota": iota,
            "ident": ident,
            "wcat": wcat,
            "brep": brep,
            "arep": arep,
            "loopn": prep["loopn"][c],
            "xts": xts[c],
        })
    return maps


def _assemble_out(cfg, results, gslot):
    yperm = np.concatenate([results[c]["out"] for c in range(cfg.ncores)],
                           axis=0)
    return np.ascontiguousarray(yperm[gslot]).astype(np.float32)


def run(cfg, x, edge_index, edge_weight, W1, b1, W2, b2, W3, b3, W4, b4,
        prelu_a, return_nc=False):
    prep = _preprocess(cfg, edge_index, edge_weight)
    nc = _build(cfg, prep["T"])
    in_maps = _make_in_maps(cfg, prep, x,
                            [np.asarray(W1, np.float32), np.asarray(W2, np.float32),
                             np.asarray(W3, np.float32), np.asarray(W4, np.float32)],
                            [np.asarray(b1, np.float32), np.asarray(b2, np.float32),
                             np.asarray(b3, np.float32), np.asarray(b4, np.float32)],
                            np.asarray(prelu_a, np.float32))
    res = run_bass_kernel_spmd(nc, in_maps, core_ids=list(range(cfg.ncores)))
    y = _assemble_out(cfg, res.results, prep["gslot"])
    if return_nc:
        return y, nc, in_maps
    return y


def kernel(x, edge_index, edge_weight, W1, b1, W2, b2, W3, b3, W4, b4, prelu_a):
    return run(CFG, x, edge_index, edge_weight,
               W1, b1, W2, b2, W3, b3, W4, b4, prelu_a)


# revision 9
# speedup vs baseline: 1.0181x; 1.0109x over previous
"""4-layer GCN encoder on 8 Trainium2 NeuronCores — ReduceScatter design.

Strategy (source-side partial aggregation):
  - Nodes are packed into 416 dst blocks of 128 by a two-phase balancer:
    cores by in-degree round-robin, then per-core greedy bin packing
    (squared-norm objective over per-source-core edge counts) so every
    (src core, dst block) bucket fits 2 edge tiles (1 for the last
    `t1blk` blocks of each core).  Core c owns blocks [c*52, (c+1)*52).
  - Edges are assigned to the core owning their SOURCE node.  Each
    layer, every core projects only its local shard (a = h @ W, kept in
    local DRAM) and gathers messages from that 3.4MB local table (int16
    row ids, no AllGather before aggregation).
  - Scatter-add on TensorE: pb[dst,f] += S^T M per 128-edge tile, where
    S = one-hot * norm.  S is precomputed on the host (graph-only data,
    passed as an input); on device half the batches are streamed from
    DRAM and half rebuilt on DVE (smod), balancing the DMA engines
    against DVE.  PSUM evacuations are split DVE/Act (evmod).
  - Partial sums are staged to SBUF and written as bf16 partials; three
    chunked ReduceScatters (28/16/8 blocks per core) sum them across
    cores, each core receiving exactly its shard rows.  RS chunks are
    emitted a few buckets into the next segment (rsd) so their input
    waits don't stall the Pool gather stream; chunks A and B hide fully
    under the body, only the small C chunk is exposed.  RS-dependent
    epilogue DMAs go through SWDGE so they cannot alias-block HWDGE
    semaphore lanes used by body/prefetch DMAs.
  - Epilogue (rs + loopnorm*a_own + b, PReLU on the last layer) is
    batched per 4 blocks; the dense for the next layer runs inside it
    and lands in wide tiles flushed with one DMA per chunk.
"""

import numpy as np
import ml_dtypes

import concourse.bacc as bacc
import concourse.mybir as mybir
import concourse.tile as tile
from concourse.bass_utils import run_bass_kernel_spmd

P = 128
BF16 = mybir.dt.bfloat16
F32 = mybir.dt.float32
I16 = mybir.dt.int16


class Cfg:
    def __init__(self, n_nodes=50000, n_edges=800000, in_ch=512, hid=256,
                 ncores=8, nb=52, segs=(28, 16, 8), ct=8, gstage=8, rsd=3,
                 smod=2, evmod=3, t1blk=4):
        self.n_nodes = n_nodes
        self.n_edges = n_edges
        self.in_ch = in_ch
        self.hid = hid
        self.ncores = ncores
        self.nb = nb                      # dst blocks per core
        self.segs = tuple(segs)           # RS chunk sizes (blocks per core)
        assert sum(segs) == nb
        self.nbt = ncores * nb            # global dst blocks
        self.shard = nb * P               # nodes per core (padded)
        self.npad = self.nbt * P
        assert self.npad >= n_nodes
        assert self.shard < 32768         # int16 gather ids
        self.ct = ct                      # tiles per gather batch
        self.gstage = gstage              # dst blocks per staging flush
        self.rsd = rsd                    # B buckets emitted before RS_A
        self.smod = smod                  # 1/smod of S batches built on DVE
        self.evmod = evmod                # 1/evmod of evacs on Act (0=none)
        self.t1blk = t1blk                # single-tile blocks per core
        for sg in segs:
            assert (ncores * sg) % gstage == 0 and sg % 4 == 0
        self.fc_in = in_ch // P
        self.fh = hid // P


CFG = Cfg()


class Buckets:
    """Uniform (across cores) bucket schedule, in processing order."""

    def __init__(self, tb_ord, nsegs):
        self.tb = list(tb_ord)            # tiles per bucket, processing order
        self.nsegs = list(nsegs)          # buckets per RS segment
        self.off = np.zeros(len(self.tb) + 1, np.int64)
        self.off[1:] = np.cumsum(self.tb)
        self.ns = int(self.off[-1])

    def __repr__(self):
        return f"Buckets(n={len(self.tb)}, NS={self.ns}, segs={self.nsegs})"


# ----------------------------------------------------------------- host prep

def _preprocess(cfg, edge_index, edge_weight):
    N = cfg.n_nodes
    src = np.asarray(edge_index[0], dtype=np.int64)
    dst = np.asarray(edge_index[1], dtype=np.int64)
    ew = np.asarray(edge_weight, dtype=np.float32)
    deg = np.bincount(dst, weights=ew.astype(np.float64), minlength=N)
    deg = deg.astype(np.float32) + 1.0
    dis = (1.0 / np.sqrt(deg)).astype(np.float32)
    norm = dis[src] * ew * dis[dst]
    loopnorm = dis * dis

    # two-phase balanced assignment: cores by in-degree round-robin, then
    # per-core greedy (squared-norm objective) bin packing of nodes into
    # blocks so that per-(src core, dst block) edge counts stay <= 2 tiles
    degc = np.bincount(dst, minlength=N)
    order = np.argsort(-degc, kind="stable")
    core_of = np.empty(N, np.int64)
    core_of[order] = np.arange(N) % cfg.ncores
    kmat = np.zeros((N, cfg.ncores), np.int32)
    np.add.at(kmat, (dst, core_of[src]), 1)
    gslot = np.empty(N, dtype=np.int64)
    caps = np.full(cfg.nb, 2 * P, np.float64)
    caps[-cfg.t1blk:] = P            # small blocks: single-tile buckets
    for c in range(cfg.ncores):
        nodes = order[core_of[order] == c]
        loads = np.zeros((cfg.nb, cfg.ncores), np.float64)
        fill = np.zeros(cfg.nb, np.int64)
        for v in nodes:
            kv = kmat[v].astype(np.float64)
            score = loads @ kv
            over = (loads + kv[None, :]).max(axis=1) > caps
            score[over] += 1e12
            score[fill >= P] = 1e18
            b = int(np.argmin(score))
            loads[b] += kv
            gslot[v] = (c * cfg.nb + b) * P + fill[b]
            fill[b] += 1

    ps = gslot[src]
    pd = gslot[dst]
    core_e = ps // cfg.shard              # edge -> owning core (by src)
    gb = pd // P                          # global dst block

    # processing order of buckets: segment-major, core-major within segment
    c_dst = gb // cfg.nb
    bl = gb % cfg.nb
    seg_of = np.zeros(cfg.nb, np.int64)
    seg_start = np.zeros(cfg.nb, np.int64)
    seg_base = np.zeros(len(cfg.segs), np.int64)
    s0 = 0
    base = 0
    for si, sg in enumerate(cfg.segs):
        seg_of[s0:s0 + sg] = si
        seg_start[s0:s0 + sg] = s0
        seg_base[si] = base
        base += cfg.ncores * sg
        s0 += sg
    si_e = seg_of[bl]
    seglen_e = np.array(cfg.segs)[si_e]
    ordblk_e = (seg_base[si_e] + c_dst * seglen_e + (bl - seg_start[bl]))

    cnt = np.zeros((cfg.ncores, cfg.nbt), np.int64)
    np.add.at(cnt, (core_e, ordblk_e), 1)
    tb_ord = np.maximum((cnt.max(axis=0) + P - 1) // P, 1)
    bk = Buckets(tb_ord, [cfg.ncores * sg for sg in cfg.segs])
    nsp = ((bk.ns + cfg.ct - 1) // cfg.ct) * cfg.ct

    srt = np.lexsort((ps, ordblk_e, core_e))
    cs, os_ = core_e[srt], ordblk_e[srt]
    key = cs * cfg.nbt + os_
    uniq, inv, counts = np.unique(key, return_inverse=True, return_counts=True)
    starts = np.zeros_like(counts)
    starts[1:] = np.cumsum(counts)[:-1]
    rank = np.arange(len(srt)) - starts[inv]
    assert (rank < tb_ord[os_] * P).all()

    slot = bk.off[os_] + rank // P
    q = slot * P + rank % P
    gidx16 = np.zeros((cfg.ncores, 16, nsp * 8), dtype=np.int16)
    dstc = np.zeros((cfg.ncores, P, nsp), dtype=np.float32)
    normc = np.zeros((cfg.ncores, P, nsp), dtype=np.float32)
    sfull = np.zeros((cfg.ncores, P, nsp, P), dtype=ml_dtypes.bfloat16)
    idxval = (ps[srt] - cs * cfg.shard).astype(np.int16)
    dloci = (pd[srt] % P).astype(np.int64)
    dlocal = dloci.astype(np.float32)
    nval = norm[srt]
    for c in range(cfg.ncores):
        m = cs == c
        qc = q[m]
        gidx16[c, qc % 16, qc // 16] = idxval[m]
        dstc[c, qc % P, qc // P] = dlocal[m]
        normc[c, qc % P, qc // P] = nval[m]
        # host-built one-hot*norm scatter matrices (S), streamed on device
        sfull[c, qc % P, qc // P, dloci[m]] = nval[m]
    gidx = np.tile(gidx16, (1, 8, 1))

    lpad = np.zeros(cfg.npad, np.float32)
    lpad[gslot] = loopnorm
    loopn = lpad.reshape(cfg.ncores, cfg.nb, P).transpose(0, 2, 1)
    return dict(T=bk, nsp=nsp, gidx=gidx,
                dstc=dstc.astype(ml_dtypes.bfloat16),
                normc=normc.astype(ml_dtypes.bfloat16),
                sfull=sfull.reshape(cfg.ncores, P, nsp * P),
                loopn=np.ascontiguousarray(loopn),
                gslot=gslot)


def _pack_xts(cfg, x, gslot):
    xpad = np.zeros((cfg.npad, cfg.in_ch), dtype=np.float32)
    xpad[gslot] = x
    a = xpad.reshape(cfg.ncores, cfg.nb, P, cfg.fc_in, P)
    a = a.transpose(0, 1, 4, 3, 2).reshape(cfg.ncores, cfg.nb * P, cfg.fc_in * P)
    return np.ascontiguousarray(a.astype(ml_dtypes.bfloat16))


def _pack_wcat(cfg, Ws):
    cols = []
    for Wl in Ws:
        k = Wl.shape[0]
        for fc in range(k // P):
            cols.append(Wl[fc * P:(fc + 1) * P, :])
    return np.concatenate(cols, axis=1).astype(ml_dtypes.bfloat16)


def _iota_np():
    return np.tile(np.arange(P, dtype=np.float32)[None, :], (P, 1)).astype(
        ml_dtypes.bfloat16)


# ----------------------------------------------------------------- builder

def _build(cfg, bk, n_layers=4):
    nsp = ((bk.ns + cfg.ct - 1) // cfg.ct) * cfg.ct
    CT = cfg.ct
    HID = cfg.hid
    GS = cfg.gstage
    seg_rows = [n * P for n in bk.nsegs]
    nc = bacc.Bacc("TRN2", target_bir_lowering=False, debug=False,
                   num_devices=cfg.ncores, num_swdge_queues=4)
    qctr = [0]

    gidx_d = nc.dram_tensor("gidx", [P, nsp * 8], I16, kind="ExternalInput")
    sdump_d = nc.dram_tensor("sdump", [P, nsp * P], BF16, kind="ExternalInput")
    dstc_d = nc.dram_tensor("dstc", [P, nsp], BF16, kind="ExternalInput")
    normc_d = nc.dram_tensor("normc", [P, nsp], BF16, kind="ExternalInput")
    iota_d = nc.dram_tensor("iota", [P, P], BF16, kind="ExternalInput")
    ident_d = nc.dram_tensor("ident", [P, P], BF16, kind="ExternalInput")
    wcat_cols = (cfg.fc_in + (n_layers - 1) * cfg.fh) * HID
    wcat_d = nc.dram_tensor("wcat", [P, wcat_cols], BF16, kind="ExternalInput")
    brep_d = nc.dram_tensor("brep", [P, n_layers * HID], F32,
                            kind="ExternalInput")
    arep_d = nc.dram_tensor("arep", [P, HID], F32, kind="ExternalInput")
    loopn_d = nc.dram_tensor("loopn", [P, cfg.nb], F32, kind="ExternalInput")
    xts_d = nc.dram_tensor("xts", [cfg.nb * P, cfg.fc_in * P], BF16,
                           kind="ExternalInput")
    out_d = nc.dram_tensor("out", [cfg.nb * P, HID], F32,
                           kind="ExternalOutput")

    GE_EPI = 4
    w_off = {}
    off = 0
    for l in range(n_layers):
        k = cfg.fc_in if l == 0 else cfg.fh
        for fc in range(k):
            w_off[(l, fc)] = off
            off += HID

    with tile.TileContext(nc) as tc:
        with (
            tc.tile_pool(name="res", bufs=1) as res,
            tc.tile_pool(name="mpool", bufs=10) as mpool,
            tc.tile_pool(name="spool", bufs=11) as spool,
            tc.tile_pool(name="xpool", bufs=6) as xpool,
            tc.tile_pool(name="apool", bufs=1) as apool,
            tc.tile_pool(name="lbpool", bufs=1) as lbpool,
            tc.tile_pool(name="hpool", bufs=3) as hpool,
            tc.tile_pool(name="htpool", bufs=1) as htpool,
            tc.tile_pool(name="stgpool", bufs=4) as stgpool,
            tc.tile_pool(name="opool", bufs=2) as opool,
            tc.tile_pool(name="ppool", bufs=4, space="PSUM") as ppool,
            tc.tile_pool(name="tpool", bufs=2, space="PSUM") as tpool,
            tc.tile_pool(name="dpool", bufs=2, space="PSUM") as dpool,
            tc.tile_pool(name="dram", bufs=2, space="DRAM") as dram,
            tc.tile_pool(name="drp", bufs=2, space="DRAM") as drp,
            tc.tile_pool(name="drs", bufs=2, space="DRAM") as drs,
        ):
            # ---- resident loads
            gidx = res.tile([P, nsp * 8], I16)
            nc.sync.dma_start(out=gidx[:], in_=gidx_d[:])
            dstc = res.tile([P, nsp], BF16)
            nc.sync.dma_start(out=dstc[:], in_=dstc_d[:])
            normc = res.tile([P, nsp], BF16)
            nc.sync.dma_start(out=normc[:], in_=normc_d[:])
            iota = res.tile([P, P], BF16)
            nc.sync.dma_start(out=iota[:], in_=iota_d[:])
            ident = res.tile([P, P], BF16)
            nc.sync.dma_start(out=ident[:], in_=ident_d[:])
            wcat = res.tile([P, wcat_cols], BF16)
            nc.sync.dma_start(out=wcat[:], in_=wcat_d[:])
            brep = res.tile([P, n_layers * HID], F32)
            nc.sync.dma_start(out=brep[:], in_=brep_d[:])
            arep = res.tile([P, HID], F32)
            nc.sync.dma_start(out=arep[:], in_=arep_d[:])
            loopn = res.tile([P, cfg.nb], F32)
            nc.sync.dma_start(out=loopn[:], in_=loopn_d[:])

            aown = {}
            lbias = {}
            hT = {}
            owide = [None]

            def dense_block(l, nt, alocal_t):
                pd_ = dpool.tile([P, HID], F32, tag="pd", name="pd")
                if l == 0:
                    xsl = xpool.tile([P, cfg.fc_in * P], BF16, tag="xsl",
                                     name="xsl")
                    nc.sync.dma_start(out=xsl[:],
                                      in_=xts_d[nt * P:(nt + 1) * P, :])
                    nk = cfg.fc_in
                    for fc in range(nk):
                        nc.tensor.matmul(
                            out=pd_[:],
                            lhsT=xsl[:, fc * P:(fc + 1) * P],
                            rhs=wcat[:, w_off[(0, fc)]:w_off[(0, fc)] + HID],
                            start=(fc == 0), stop=(fc == nk - 1))
                else:
                    for fc in range(cfg.fh):
                        nc.tensor.matmul(
                            out=pd_[:],
                            lhsT=hT[nt][:, fc * P:(fc + 1) * P],
                            rhs=wcat[:, w_off[(l, fc)]:w_off[(l, fc)] + HID],
                            start=(fc == 0), stop=(fc == cfg.fh - 1))
                # dense outputs land in per-chunk wide tiles: one batched
                # alocal DMA per GE blocks instead of 56 small SWDGE ops
                ch, j = nt // GE_EPI, nt % GE_EPI
                if j == 0:
                    aown[ch] = apool.tile([P, GE_EPI * HID], BF16,
                                          tag=f"aw{ch}", name=f"aw{ch}")
                asb = aown[ch][:, j * HID:(j + 1) * HID]
                nc.scalar.copy(out=asb, in_=pd_[:])
                if j == GE_EPI - 1 or nt == cfg.nb - 1:
                    n0 = nt - j
                    nc.gpsimd.dma_start(
                        out=alocal_t[n0 * P:(nt + 1) * P, :].rearrange(
                            "(g p) f -> p g f", p=P),
                        in_=aown[ch][:, :(j + 1) * HID].rearrange(
                            "p (g f) -> p g f", f=HID))
                # self-loop + bias term, off the post-RS critical path:
                # lb = loopnorm * a_own + b
                lt = hpool.tile([P, HID], F32, tag="lt", name="lt")
                nc.vector.tensor_scalar(
                    out=lt[:], in0=asb,
                    scalar1=loopn[:, nt:nt + 1], scalar2=None,
                    op0=mybir.AluOpType.mult)
                lb = lbpool.tile([P, HID], BF16, tag=f"lb{nt}",
                                 name=f"lb{nt}")
                nc.vector.tensor_tensor(
                    out=lb[:], in0=lt[:], in1=brep[:, l * HID:(l + 1) * HID],
                    op=mybir.AluOpType.add)
                lbias[nt] = lb

            def body(l, alocal_t, p_ts):
                """Aggregate all buckets; write partials; fire RS_A/RS_B.
                Returns (rsA_t, rsB_t)."""
                batches = {}

                def get_batch(bi):
                    if bi in batches:
                        return batches[bi]
                    k0 = bi * CT
                    M = mpool.tile([P, CT * HID], BF16, tag="M", name="M")
                    nc.gpsimd.dma_gather(
                        out_ap=M[:].rearrange("p (t e) -> p t e", e=HID),
                        in_ap=alocal_t[:],
                        idxs_ap=gidx[:, k0 * 8:(k0 + CT) * 8],
                        num_idxs=CT * P,
                        num_idxs_reg=CT * P,
                        elem_size=HID,
                        queue_num=qctr[0] % 3,
                    )
                    qctr[0] += 1
                    S = spool.tile([P, CT * P], BF16, tag="S", name="S")
                    build = cfg.smod > 0 and bi % cfg.smod == cfg.smod - 1
                    if build:
                        # rebuild on DVE to relieve the DMA engines
                        s3 = S[:].rearrange("p (t e) -> p t e", e=P)
                        iob = iota[:].rearrange(
                            "p (o e) -> p o e", o=1).broadcast_to([P, CT, P])
                        nc.vector.tensor_tensor(
                            out=s3, in0=iob,
                            in1=dstc[:, k0:k0 + CT].to_broadcast([P, CT, P]),
                            op=mybir.AluOpType.is_equal)
                        nc.vector.tensor_tensor(
                            out=s3, in0=s3,
                            in1=normc[:, k0:k0 + CT].to_broadcast([P, CT, P]),
                            op=mybir.AluOpType.mult)
                    else:
                        nc.sync.dma_start(
                            out=S[:], in_=sdump_d[:, k0 * P:(k0 + CT) * P])
                    batches[bi] = (M, S)
                    return batches[bi]

                def emit_rs(seg, p_t, rows):
                    rs_t = drs.tile([rows // cfg.ncores, HID], BF16,
                                    tag=f"rs{seg}", name=f"rs{seg}")
                    nc.gpsimd.collective_compute(
                        "ReduceScatter",
                        mybir.AluOpType.add,
                        ins=[p_t[:].opt()],
                        outs=[rs_t[:].opt()],
                        replica_groups=[list(range(cfg.ncores))],
                    )
                    return rs_t

                nsg = len(bk.nsegs)
                rs = [None] * nsg
                stg = None
                base = 0
                for seg in range(nsg):
                    nseg = bk.nsegs[seg]
                    p_t = p_ts[seg]
                    for i in range(nseg):
                        if seg > 0 and i == cfg.rsd:
                            # emit the previous segment's RS only after a few
                            # gather batches of this segment are queued on
                            # Pool, so its input wait doesn't stall the
                            # gather stream
                            rs[seg - 1] = emit_rs(seg - 1, p_ts[seg - 1],
                                                  seg_rows[seg - 1])
                        bseq = base + i
                        off = int(bk.off[bseq])
                        tbn = int(bk.tb[bseq])
                        pb = ppool.tile([P, HID], F32, tag="pb", name="pb")
                        for t in range(tbn):
                            s = off + t
                            M, S = get_batch(s // CT)
                            j = s % CT
                            nc.tensor.matmul(
                                out=pb[:],
                                lhsT=S[:, j * P:(j + 1) * P],
                                rhs=M[:, j * HID:(j + 1) * HID],
                                start=(t == 0), stop=(t == tbn - 1))
                        gpos = i % GS
                        if gpos == 0:
                            stg = stgpool.tile([P, GS * HID], BF16, tag="stg",
                                               name="stg")
                        # evac PSUM->SBUF: DVE when idle (l>0), Act on l0
                        # (DVE builds S there); evmod shifts some to Act
                        on_act = (cfg.evmod > 0 and i % cfg.evmod == 1)
                        if not on_act:
                            nc.vector.tensor_scalar(
                                out=stg[:, gpos * HID:(gpos + 1) * HID],
                                in0=pb[:], scalar1=0.0, scalar2=None,
                                op0=mybir.AluOpType.add)
                        else:
                            nc.scalar.copy(
                                out=stg[:, gpos * HID:(gpos + 1) * HID],
                                in_=pb[:])
                        if gpos == GS - 1:
                            g0 = i - gpos
                            view = p_t[g0 * P:(g0 + GS) * P, :].rearrange(
                                "(g p) f -> p g f", p=P)
                            nc.sync.dma_start(
                                out=view,
                                in_=stg[:].rearrange("p (g f) -> p g f",
                                                     f=HID))
                    base += nseg
                    if seg == nsg - 1:
                        if rs[seg - 1] is None:
                            rs[seg - 1] = emit_rs(seg - 1, p_ts[seg - 1],
                                                  seg_rows[seg - 1])
                        rs[seg] = emit_rs(seg, p_t, seg_rows[seg])
                return rs

            def epilogue_block(l, nt, rsr, alocal_next):
                if l < n_layers - 1:
                    hsb = hpool.tile([P, HID], BF16, tag="hsb", name="hsb")
                    nc.vector.tensor_tensor(
                        out=hsb[:], in0=rsr, in1=lbias[nt][:],
                        op=mybir.AluOpType.add)
                    tp = tpool.tile([P, 2 * P], BF16, tag="tp", name="tp")
                    for fh in range(cfg.fh):
                        nc.tensor.transpose(
                            tp[:, fh * P:(fh + 1) * P],
                            hsb[:, fh * P:(fh + 1) * P], ident[:])
                    ht = htpool.tile([P, 2 * P], BF16, tag=f"hT{nt}",
                                     name=f"hT{nt}")
                    nc.scalar.copy(out=ht[:], in_=tp[:])
                    hT[nt] = ht
                    dense_block(l + 1, nt, alocal_next)
                else:
                    ve = nc.gpsimd if nt % 3 == 2 else nc.vector
                    hb2 = opool.tile([P, HID], F32, tag="hb2", name="hb2")
                    ve.tensor_tensor(
                        out=hb2[:], in0=rsr, in1=lbias[nt][:],
                        op=mybir.AluOpType.add)
                    t1 = opool.tile([P, HID], F32, tag="t1", name="t1")
                    ve.tensor_scalar(
                        out=t1[:], in0=hb2[:], scalar1=0.0, scalar2=None,
                        op0=mybir.AluOpType.min)
                    ve.tensor_tensor(
                        out=t1[:], in0=t1[:], in1=arep[:],
                        op=mybir.AluOpType.mult)
                    ch, j = nt // GE_EPI, nt % GE_EPI
                    if j == 0:
                        owide[0] = opool.tile([P, GE_EPI * HID], BF16,
                                              tag="ow", name="ow")
                    osl = owide[0][:, j * HID:(j + 1) * HID]
                    ve.tensor_scalar(
                        out=osl, in0=hb2[:], scalar1=0.0, scalar2=None,
                        op0=mybir.AluOpType.max)
                    ve.tensor_tensor(
                        out=osl, in0=osl, in1=t1[:],
                        op=mybir.AluOpType.add)
                    if j == GE_EPI - 1 or nt == cfg.nb - 1:
                        n0 = nt - j
                        # SWDGE cast bf16->f32 on the way out
                        nc.gpsimd.dma_start(
                            out=out_d[n0 * P:(nt + 1) * P, :].rearrange(
                                "(g p) f -> p g f", p=P),
                            in_=owide[0][:, :(j + 1) * HID].rearrange(
                                "p (g f) -> p g f", f=HID))

            # ---- layer pipeline
            alocal = dram.tile([cfg.shard, HID], BF16, tag="alocal",
                               name="alocal")
            for nt in range(cfg.nb):
                dense_block(0, nt, alocal)
            seg_lo = []
            s0 = 0
            for sg in cfg.segs:
                seg_lo.append(s0)
                s0 += sg
            for l in range(n_layers):
                p_ts = [drp.tile([r, HID], BF16, tag=f"p{si}", name=f"p{si}")
                        for si, r in enumerate(seg_rows)]
                rs_ts = body(l, alocal, p_ts)
                if l < n_layers - 1:
                    alocal = dram.tile([cfg.shard, HID], BF16, tag="alocal",
                                       name="alocal")
                # epilogues in chunks of GE blocks: one batched rs load each
                GE = GE_EPI
                for nt0 in range(0, cfg.nb, GE):
                    ng = min(GE, cfg.nb - nt0)
                    si = max(i for i in range(len(seg_lo))
                             if seg_lo[i] <= nt0)
                    assert nt0 + ng <= seg_lo[si] + cfg.segs[si]
                    rs_t, row0 = rs_ts[si], (nt0 - seg_lo[si]) * P
                    rsc = hpool.tile([P, GE * HID], BF16, tag="rsc",
                                     name="rsc")
                    nc.gpsimd.dma_start(
                        out=rsc[:, :ng * HID].rearrange("p (g f) -> p g f",
                                                        f=HID),
                        in_=rs_t[row0:row0 + ng * P, :].rearrange(
                            "(g p) f -> p g f", p=P))
                    for j in range(ng):
                        epilogue_block(l, nt0 + j,
                                       rsc[:, (j) * HID:(j + 1) * HID],
                                       alocal)

    nc.compile()
    return nc


# ----------------------------------------------------------------- execution

def _make_in_maps(cfg, prep, x, Ws, bs, prelu_a):
    xts = _pack_xts(cfg, np.asarray(x, np.float32), prep["gslot"])
    wcat = _pack_wcat(cfg, Ws)
    brep = np.zeros((P, 4 * cfg.hid), np.float32)
    for l, b in enumerate(bs):
        brep[:, l * cfg.hid:(l + 1) * cfg.hid] = b[None, :]
    arep = np.tile(np.asarray(prelu_a, np.float32)[None, :], (P, 1))
    iota = _iota_np()
    ident = np.eye(P, dtype=ml_dtypes.bfloat16)
    maps = []
    for c in range(cfg.ncores):
        maps.append({
            "gidx": prep["gidx"][c],
            "sdump": prep["sfull"][c],
            "dstc": prep["dstc"][c],
            "normc": prep["normc"][c],
            "iota": iota,
            "ident": ident,
            "wcat": wcat,
            "brep": brep,
            "arep": arep,
            "loopn": prep["loopn"][c],
            "xts": xts[c],
        })
    return maps


def _assemble_out(cfg, results, gslot):
    yperm = np.concatenate([results[c]["out"] for c in range(cfg.ncores)],
                           axis=0)
    return np.ascontiguousarray(yperm[gslot]).astype(np.float32)


def run(cfg, x, edge_index, edge_weight, W1, b1, W2, b2, W3, b3, W4, b4,
        prelu_a, return_nc=False):
    prep = _preprocess(cfg, edge_index, edge_weight)
    nc = _build(cfg, prep["T"])
    in_maps = _make_in_maps(cfg, prep, x,
                            [np.asarray(W1, np.float32), np.asarray(W2, np.float32),
                             np.asarray(W3, np.float32), np.asarray(W4, np.float32)],
                            [np.asarray(b1, np.float32), np.asarray(b2, np.float32),
                             np.asarray(b3, np.float32), np.asarray(b4, np.float32)],
                            np.asarray(prelu_a, np.float32))
    res = run_bass_kernel_spmd(nc, in_maps, core_ids=list(range(cfg.ncores)))
    y = _assemble_out(cfg, res.results, prep["gslot"])
    if return_nc:
        return y, nc, in_maps
    return y


def kernel(x, edge_index, edge_weight, W1, b1, W2, b2, W3, b3, W4, b4, prelu_a):
    return run(CFG, x, edge_index, edge_weight,
               W1, b1, W2, b2, W3, b3, W4, b4, prelu_a)


# revision 10
# speedup vs baseline: 1.0195x; 1.0014x over previous
"""4-layer GCN encoder on 8 Trainium2 NeuronCores — ReduceScatter design.

Strategy (source-side partial aggregation):
  - Nodes are packed into 416 dst blocks of 128 by a two-phase balancer:
    cores by in-degree round-robin, then per-core greedy bin packing
    (squared-norm objective over per-source-core edge counts) so every
    (src core, dst block) bucket fits 2 edge tiles (1 for the last
    `t1blk` blocks of each core).  Core c owns blocks [c*52, (c+1)*52).
  - Edges are assigned to the core owning their SOURCE node.  Each
    layer, every core projects only its local shard (a = h @ W, kept in
    local DRAM) and gathers messages from that 3.4MB local table (int16
    row ids, no AllGather before aggregation).
  - Scatter-add on TensorE: pb[dst,f] += S^T M per 128-edge tile, where
    S = one-hot * norm.  S is precomputed on the host (graph-only data,
    passed as an input); on device half the batches are streamed from
    DRAM and half rebuilt on DVE (smod), balancing the DMA engines
    against DVE.  PSUM evacuations are split DVE/Act (evmod).
  - Partial sums are staged to SBUF and written as bf16 partials; three
    chunked ReduceScatters (28/16/8 blocks per core) sum them across
    cores, each core receiving exactly its shard rows.  RS chunks are
    emitted a few buckets into the next segment (rsd) so their input
    waits don't stall the Pool gather stream; chunks A and B hide fully
    under the body, only the small C chunk is exposed.  RS-dependent
    epilogue DMAs go through SWDGE so they cannot alias-block HWDGE
    semaphore lanes used by body/prefetch DMAs.
  - Epilogue (rs + loopnorm*a_own + b, PReLU on the last layer) is
    batched per 4 blocks; the dense for the next layer runs inside it
    and lands in wide tiles flushed with one DMA per chunk.
"""

import numpy as np
import ml_dtypes

import concourse.bacc as bacc
import concourse.mybir as mybir
import concourse.tile as tile
from concourse.bass_utils import run_bass_kernel_spmd

P = 128
BF16 = mybir.dt.bfloat16
F32 = mybir.dt.float32
I16 = mybir.dt.int16


class Cfg:
    def __init__(self, n_nodes=50000, n_edges=800000, in_ch=512, hid=256,
                 ncores=8, nb=52, segs=(28, 16, 8), ct=8, gstage=8, rsd=3,
                 smod=2, evmod=3, t1blk=4):
        self.n_nodes = n_nodes
        self.n_edges = n_edges
        self.in_ch = in_ch
        self.hid = hid
        self.ncores = ncores
        self.nb = nb                      # dst blocks per core
        self.segs = tuple(segs)           # RS chunk sizes (blocks per core)
        assert sum(segs) == nb
        self.nbt = ncores * nb            # global dst blocks
        self.shard = nb * P               # nodes per core (padded)
        self.npad = self.nbt * P
        assert self.npad >= n_nodes
        assert self.shard < 32768         # int16 gather ids
        self.ct = ct                      # tiles per gather batch
        self.gstage = gstage              # dst blocks per staging flush
        self.rsd = rsd                    # B buckets emitted before RS_A
        self.smod = smod                  # 1/smod of S batches built on DVE
        self.evmod = evmod                # 1/evmod of evacs on Act (0=none)
        self.t1blk = t1blk                # single-tile blocks per core
        for sg in segs:
            assert (ncores * sg) % gstage == 0 and sg % 4 == 0
        self.fc_in = in_ch // P
        self.fh = hid // P


CFG = Cfg()


class Buckets:
    """Uniform (across cores) bucket schedule, in processing order."""

    def __init__(self, tb_ord, nsegs):
        self.tb = list(tb_ord)            # tiles per bucket, processing order
        self.nsegs = list(nsegs)          # buckets per RS segment
        self.off = np.zeros(len(self.tb) + 1, np.int64)
        self.off[1:] = np.cumsum(self.tb)
        self.ns = int(self.off[-1])

    def __repr__(self):
        return f"Buckets(n={len(self.tb)}, NS={self.ns}, segs={self.nsegs})"


# ----------------------------------------------------------------- host prep

def _preprocess(cfg, edge_index, edge_weight):
    N = cfg.n_nodes
    src = np.asarray(edge_index[0], dtype=np.int64)
    dst = np.asarray(edge_index[1], dtype=np.int64)
    ew = np.asarray(edge_weight, dtype=np.float32)
    deg = np.bincount(dst, weights=ew.astype(np.float64), minlength=N)
    deg = deg.astype(np.float32) + 1.0
    dis = (1.0 / np.sqrt(deg)).astype(np.float32)
    norm = dis[src] * ew * dis[dst]
    loopnorm = dis * dis

    # two-phase balanced assignment: cores by in-degree round-robin, then
    # per-core greedy (squared-norm objective) bin packing of nodes into
    # blocks so that per-(src core, dst block) edge counts stay <= 2 tiles
    degc = np.bincount(dst, minlength=N)
    order = np.argsort(-degc, kind="stable")
    core_of = np.empty(N, np.int64)
    core_of[order] = np.arange(N) % cfg.ncores
    kmat = np.zeros((N, cfg.ncores), np.int32)
    np.add.at(kmat, (dst, core_of[src]), 1)
    gslot = np.empty(N, dtype=np.int64)
    caps = np.full(cfg.nb, 2 * P, np.float64)
    caps[-cfg.t1blk:] = P            # small blocks: single-tile buckets
    for c in range(cfg.ncores):
        nodes = order[core_of[order] == c]
        loads = np.zeros((cfg.nb, cfg.ncores), np.float64)
        fill = np.zeros(cfg.nb, np.int64)
        for v in nodes:
            kv = kmat[v].astype(np.float64)
            score = loads @ kv
            over = (loads + kv[None, :]).max(axis=1) > caps
            score[over] += 1e12
            score[fill >= P] = 1e18
            b = int(np.argmin(score))
            loads[b] += kv
            gslot[v] = (c * cfg.nb + b) * P + fill[b]
            fill[b] += 1

    ps = gslot[src]
    pd = gslot[dst]
    core_e = ps // cfg.shard              # edge -> owning core (by src)
    gb = pd // P                          # global dst block

    # processing order of buckets: segment-major, core-major within segment
    c_dst = gb // cfg.nb
    bl = gb % cfg.nb
    seg_of = np.zeros(cfg.nb, np.int64)
    seg_start = np.zeros(cfg.nb, np.int64)
    seg_base = np.zeros(len(cfg.segs), np.int64)
    s0 = 0
    base = 0
    for si, sg in enumerate(cfg.segs):
        seg_of[s0:s0 + sg] = si
        seg_start[s0:s0 + sg] = s0
        seg_base[si] = base
        base += cfg.ncores * sg
        s0 += sg
    si_e = seg_of[bl]
    seglen_e = np.array(cfg.segs)[si_e]
    ordblk_e = (seg_base[si_e] + c_dst * seglen_e + (bl - seg_start[bl]))

    cnt = np.zeros((cfg.ncores, cfg.nbt), np.int64)
    np.add.at(cnt, (core_e, ordblk_e), 1)
    tb_ord = np.maximum((cnt.max(axis=0) + P - 1) // P, 1)
    bk = Buckets(tb_ord, [cfg.ncores * sg for sg in cfg.segs])
    nsp = ((bk.ns + cfg.ct - 1) // cfg.ct) * cfg.ct

    srt = np.lexsort((ps, ordblk_e, core_e))
    cs, os_ = core_e[srt], ordblk_e[srt]
    key = cs * cfg.nbt + os_
    uniq, inv, counts = np.unique(key, return_inverse=True, return_counts=True)
    starts = np.zeros_like(counts)
    starts[1:] = np.cumsum(counts)[:-1]
    rank = np.arange(len(srt)) - starts[inv]
    assert (rank < tb_ord[os_] * P).all()

    slot = bk.off[os_] + rank // P
    q = slot * P + rank % P
    gidx16 = np.zeros((cfg.ncores, 16, nsp * 8), dtype=np.int16)
    dstc = np.zeros((cfg.ncores, P, nsp), dtype=np.float32)
    normc = np.zeros((cfg.ncores, P, nsp), dtype=np.float32)
    sfull = np.zeros((cfg.ncores, P, nsp, P), dtype=ml_dtypes.bfloat16)
    idxval = (ps[srt] - cs * cfg.shard).astype(np.int16)
    dloci = (pd[srt] % P).astype(np.int64)
    dlocal = dloci.astype(np.float32)
    nval = norm[srt]
    for c in range(cfg.ncores):
        m = cs == c
        qc = q[m]
        gidx16[c, qc % 16, qc // 16] = idxval[m]
        dstc[c, qc % P, qc // P] = dlocal[m]
        normc[c, qc % P, qc // P] = nval[m]
        # host-built one-hot*norm scatter matrices (S), streamed on device
        sfull[c, qc % P, qc // P, dloci[m]] = nval[m]
    gidx = np.tile(gidx16, (1, 8, 1))

    lpad = np.zeros(cfg.npad, np.float32)
    lpad[gslot] = loopnorm
    loopn = lpad.reshape(cfg.ncores, cfg.nb, P).transpose(0, 2, 1)
    return dict(T=bk, nsp=nsp, gidx=gidx,
                dstc=dstc.astype(ml_dtypes.bfloat16),
                normc=normc.astype(ml_dtypes.bfloat16),
                sfull=sfull.reshape(cfg.ncores, P, nsp * P),
                loopn=np.ascontiguousarray(loopn),
                gslot=gslot)


def _pack_xts(cfg, x, gslot):
    xpad = np.zeros((cfg.npad, cfg.in_ch), dtype=np.float32)
    xpad[gslot] = x
    a = xpad.reshape(cfg.ncores, cfg.nb, P, cfg.fc_in, P)
    a = a.transpose(0, 1, 4, 3, 2).reshape(cfg.ncores, cfg.nb * P, cfg.fc_in * P)
    return np.ascontiguousarray(a.astype(ml_dtypes.bfloat16))


def _pack_wcat(cfg, Ws):
    cols = []
    for Wl in Ws:
        k = Wl.shape[0]
        for fc in range(k // P):
            cols.append(Wl[fc * P:(fc + 1) * P, :])
    return np.concatenate(cols, axis=1).astype(ml_dtypes.bfloat16)


def _iota_np():
    return np.tile(np.arange(P, dtype=np.float32)[None, :], (P, 1)).astype(
        ml_dtypes.bfloat16)


# ----------------------------------------------------------------- builder

def _build(cfg, bk, n_layers=4):
    nsp = ((bk.ns + cfg.ct - 1) // cfg.ct) * cfg.ct
    CT = cfg.ct
    HID = cfg.hid
    GS = cfg.gstage
    seg_rows = [n * P for n in bk.nsegs]
    nc = bacc.Bacc("TRN2", target_bir_lowering=False, debug=False,
                   num_devices=cfg.ncores, num_swdge_queues=4)
    qctr = [0]

    gidx_d = nc.dram_tensor("gidx", [P, nsp * 8], I16, kind="ExternalInput")
    sdump_d = nc.dram_tensor("sdump", [P, nsp * P], BF16, kind="ExternalInput")
    dstc_d = nc.dram_tensor("dstc", [P, nsp], BF16, kind="ExternalInput")
    normc_d = nc.dram_tensor("normc", [P, nsp], BF16, kind="ExternalInput")
    iota_d = nc.dram_tensor("iota", [P, P], BF16, kind="ExternalInput")
    ident_d = nc.dram_tensor("ident", [P, P], BF16, kind="ExternalInput")
    wcat_cols = (cfg.fc_in + (n_layers - 1) * cfg.fh) * HID
    wcat_d = nc.dram_tensor("wcat", [P, wcat_cols], BF16, kind="ExternalInput")
    brep_d = nc.dram_tensor("brep", [P, n_layers * HID], F32,
                            kind="ExternalInput")
    arep_d = nc.dram_tensor("arep", [P, HID], F32, kind="ExternalInput")
    loopn_d = nc.dram_tensor("loopn", [P, cfg.nb], F32, kind="ExternalInput")
    xts_d = nc.dram_tensor("xts", [cfg.nb * P, cfg.fc_in * P], BF16,
                           kind="ExternalInput")
    out_d = nc.dram_tensor("out", [cfg.nb * P, HID], F32,
                           kind="ExternalOutput")

    GE_EPI = 4
    w_off = {}
    off = 0
    for l in range(n_layers):
        k = cfg.fc_in if l == 0 else cfg.fh
        for fc in range(k):
            w_off[(l, fc)] = off
            off += HID

    with tile.TileContext(nc) as tc:
        with (
            tc.tile_pool(name="res", bufs=1) as res,
            tc.tile_pool(name="mpool", bufs=10) as mpool,
            tc.tile_pool(name="spool", bufs=11) as spool,
            tc.tile_pool(name="xpool", bufs=6) as xpool,
            tc.tile_pool(name="apool", bufs=1) as apool,
            tc.tile_pool(name="lbpool", bufs=1) as lbpool,
            tc.tile_pool(name="hpool", bufs=3) as hpool,
            tc.tile_pool(name="htpool", bufs=1) as htpool,
            tc.tile_pool(name="stgpool", bufs=4) as stgpool,
            tc.tile_pool(name="opool", bufs=2) as opool,
            tc.tile_pool(name="ppool", bufs=4, space="PSUM") as ppool,
            tc.tile_pool(name="tpool", bufs=2, space="PSUM") as tpool,
            tc.tile_pool(name="dpool", bufs=2, space="PSUM") as dpool,
            tc.tile_pool(name="dram", bufs=2, space="DRAM") as dram,
            tc.tile_pool(name="drp", bufs=2, space="DRAM") as drp,
            tc.tile_pool(name="drs", bufs=2, space="DRAM") as drs,
        ):
            # ---- resident loads
            gidx = res.tile([P, nsp * 8], I16)
            nc.sync.dma_start(out=gidx[:], in_=gidx_d[:])
            dstc = res.tile([P, nsp], BF16)
            nc.sync.dma_start(out=dstc[:], in_=dstc_d[:])
            normc = res.tile([P, nsp], BF16)
            nc.sync.dma_start(out=normc[:], in_=normc_d[:])
            iota = res.tile([P, P], BF16)
            nc.sync.dma_start(out=iota[:], in_=iota_d[:])
            ident = res.tile([P, P], BF16)
            nc.sync.dma_start(out=ident[:], in_=ident_d[:])
            wcat = res.tile([P, wcat_cols], BF16)
            nc.sync.dma_start(out=wcat[:], in_=wcat_d[:])
            brep = res.tile([P, n_layers * HID], F32)
            nc.sync.dma_start(out=brep[:], in_=brep_d[:])
            arep = res.tile([P, HID], F32)
            nc.sync.dma_start(out=arep[:], in_=arep_d[:])
            loopn = res.tile([P, cfg.nb], F32)
            nc.sync.dma_start(out=loopn[:], in_=loopn_d[:])

            aown = {}
            lbias = {}
            hT = {}
            owide = [None]

            def dense_block(l, nt, alocal_t):
                pd_ = dpool.tile([P, HID], F32, tag="pd", name="pd")
                if l == 0:
                    xsl = xpool.tile([P, cfg.fc_in * P], BF16, tag="xsl",
                                     name="xsl")
                    nc.sync.dma_start(out=xsl[:],
                                      in_=xts_d[nt * P:(nt + 1) * P, :])
                    nk = cfg.fc_in
                    for fc in range(nk):
                        nc.tensor.matmul(
                            out=pd_[:],
                            lhsT=xsl[:, fc * P:(fc + 1) * P],
                            rhs=wcat[:, w_off[(0, fc)]:w_off[(0, fc)] + HID],
                            start=(fc == 0), stop=(fc == nk - 1))
                else:
                    for fc in range(cfg.fh):
                        nc.tensor.matmul(
                            out=pd_[:],
                            lhsT=hT[nt][:, fc * P:(fc + 1) * P],
                            rhs=wcat[:, w_off[(l, fc)]:w_off[(l, fc)] + HID],
                            start=(fc == 0), stop=(fc == cfg.fh - 1))
                # dense outputs land in per-chunk wide tiles: one batched
                # alocal DMA per GE blocks instead of 56 small SWDGE ops
                ch, j = nt // GE_EPI, nt % GE_EPI
                if j == 0:
                    aown[ch] = apool.tile([P, GE_EPI * HID], BF16,
                                          tag=f"aw{ch}", name=f"aw{ch}")
                asb = aown[ch][:, j * HID:(j + 1) * HID]
                nc.scalar.copy(out=asb, in_=pd_[:])
                if j == GE_EPI - 1 or nt == cfg.nb - 1:
                    n0 = nt - j
                    nc.gpsimd.dma_start(
                        out=alocal_t[n0 * P:(nt + 1) * P, :].rearrange(
                            "(g p) f -> p g f", p=P),
                        in_=aown[ch][:, :(j + 1) * HID].rearrange(
                            "p (g f) -> p g f", f=HID))
                # self-loop + bias term, off the post-RS critical path:
                # lb = loopnorm * a_own + b
                lt = hpool.tile([P, HID], F32, tag="lt", name="lt")
                nc.vector.tensor_scalar(
                    out=lt[:], in0=asb,
                    scalar1=loopn[:, nt:nt + 1], scalar2=None,
                    op0=mybir.AluOpType.mult)
                lb = lbpool.tile([P, HID], BF16, tag=f"lb{nt}",
                                 name=f"lb{nt}")
                nc.vector.tensor_tensor(
                    out=lb[:], in0=lt[:], in1=brep[:, l * HID:(l + 1) * HID],
                    op=mybir.AluOpType.add)
                lbias[nt] = lb

            def body(l, alocal_t, p_ts):
                """Aggregate all buckets; write partials; fire RS_A/RS_B.
                Returns (rsA_t, rsB_t)."""
                batches = {}

                def get_batch(bi):
                    if bi in batches:
                        return batches[bi]
                    k0 = bi * CT
                    M = mpool.tile([P, CT * HID], BF16, tag="M", name="M")
                    nc.gpsimd.dma_gather(
                        out_ap=M[:].rearrange("p (t e) -> p t e", e=HID),
                        in_ap=alocal_t[:],
                        idxs_ap=gidx[:, k0 * 8:(k0 + CT) * 8],
                        num_idxs=CT * P,
                        num_idxs_reg=CT * P,
                        elem_size=HID,
                        queue_num=qctr[0] % 3,
                    )
                    qctr[0] += 1
                    S = spool.tile([P, CT * P], BF16, tag="S", name="S")
                    # first 11 batches stream (prefetchable into the preceding neck
                    # via the deep S pool); DVE builds concentrate later
                    build = bi >= 11 and (bi - 11) % 7 < 4
                    if build:
                        # rebuild on DVE to relieve the DMA engines
                        s3 = S[:].rearrange("p (t e) -> p t e", e=P)
                        iob = iota[:].rearrange(
                            "p (o e) -> p o e", o=1).broadcast_to([P, CT, P])
                        nc.vector.tensor_tensor(
                            out=s3, in0=iob,
                            in1=dstc[:, k0:k0 + CT].to_broadcast([P, CT, P]),
                            op=mybir.AluOpType.is_equal)
                        nc.vector.tensor_tensor(
                            out=s3, in0=s3,
                            in1=normc[:, k0:k0 + CT].to_broadcast([P, CT, P]),
                            op=mybir.AluOpType.mult)
                    else:
                        nc.sync.dma_start(
                            out=S[:], in_=sdump_d[:, k0 * P:(k0 + CT) * P])
                    batches[bi] = (M, S)
                    return batches[bi]

                def emit_rs(seg, p_t, rows):
                    rs_t = drs.tile([rows // cfg.ncores, HID], BF16,
                                    tag=f"rs{seg}", name=f"rs{seg}")
                    nc.gpsimd.collective_compute(
                        "ReduceScatter",
                        mybir.AluOpType.add,
                        ins=[p_t[:].opt()],
                        outs=[rs_t[:].opt()],
                        replica_groups=[list(range(cfg.ncores))],
                    )
                    return rs_t

                nsg = len(bk.nsegs)
                rs = [None] * nsg
                stg = None
                base = 0
                for seg in range(nsg):
                    nseg = bk.nsegs[seg]
                    p_t = p_ts[seg]
                    for i in range(nseg):
                        if seg > 0 and i == cfg.rsd:
                            # emit the previous segment's RS only after a few
                            # gather batches of this segment are queued on
                            # Pool, so its input wait doesn't stall the
                            # gather stream
                            rs[seg - 1] = emit_rs(seg - 1, p_ts[seg - 1],
                                                  seg_rows[seg - 1])
                        bseq = base + i
                        off = int(bk.off[bseq])
                        tbn = int(bk.tb[bseq])
                        pb = ppool.tile([P, HID], F32, tag="pb", name="pb")
                        for t in range(tbn):
                            s = off + t
                            M, S = get_batch(s // CT)
                            j = s % CT
                            nc.tensor.matmul(
                                out=pb[:],
                                lhsT=S[:, j * P:(j + 1) * P],
                                rhs=M[:, j * HID:(j + 1) * HID],
                                start=(t == 0), stop=(t == tbn - 1))
                        gpos = i % GS
                        if gpos == 0:
                            stg = stgpool.tile([P, GS * HID], BF16, tag="stg",
                                               name="stg")
                        # evac PSUM->SBUF: DVE when idle (l>0), Act on l0
                        # (DVE builds S there); evmod shifts some to Act
                        on_act = (cfg.evmod > 0 and i % cfg.evmod == 1)
                        if not on_act:
                            nc.vector.tensor_scalar(
                                out=stg[:, gpos * HID:(gpos + 1) * HID],
                                in0=pb[:], scalar1=0.0, scalar2=None,
                                op0=mybir.AluOpType.add)
                        else:
                            nc.scalar.copy(
                                out=stg[:, gpos * HID:(gpos + 1) * HID],
                                in_=pb[:])
                        if gpos == GS - 1:
                            g0 = i - gpos
                            view = p_t[g0 * P:(g0 + GS) * P, :].rearrange(
                                "(g p) f -> p g f", p=P)
                            nc.sync.dma_start(
                                out=view,
                                in_=stg[:].rearrange("p (g f) -> p g f",
                                                     f=HID))
                    base += nseg
                    if seg == nsg - 1:
                        if rs[seg - 1] is None:
                            rs[seg - 1] = emit_rs(seg - 1, p_ts[seg - 1],
                                                  seg_rows[seg - 1])
                        rs[seg] = emit_rs(seg, p_t, seg_rows[seg])
                return rs

            def epilogue_block(l, nt, rsr, alocal_next):
                if l < n_layers - 1:
                    hsb = hpool.tile([P, HID], BF16, tag="hsb", name="hsb")
                    nc.vector.tensor_tensor(
                        out=hsb[:], in0=rsr, in1=lbias[nt][:],
                        op=mybir.AluOpType.add)
                    tp = tpool.tile([P, 2 * P], BF16, tag="tp", name="tp")
                    for fh in range(cfg.fh):
                        nc.tensor.transpose(
                            tp[:, fh * P:(fh + 1) * P],
                            hsb[:, fh * P:(fh + 1) * P], ident[:])
                    ht = htpool.tile([P, 2 * P], BF16, tag=f"hT{nt}",
                                     name=f"hT{nt}")
                    nc.scalar.copy(out=ht[:], in_=tp[:])
                    hT[nt] = ht
                    dense_block(l + 1, nt, alocal_next)
                else:
                    ve = nc.gpsimd if nt % 3 == 2 else nc.vector
                    hb2 = opool.tile([P, HID], F32, tag="hb2", name="hb2")
                    ve.tensor_tensor(
                        out=hb2[:], in0=rsr, in1=lbias[nt][:],
                        op=mybir.AluOpType.add)
                    t1 = opool.tile([P, HID], F32, tag="t1", name="t1")
                    ve.tensor_scalar(
                        out=t1[:], in0=hb2[:], scalar1=0.0, scalar2=None,
                        op0=mybir.AluOpType.min)
                    ve.tensor_tensor(
                        out=t1[:], in0=t1[:], in1=arep[:],
                        op=mybir.AluOpType.mult)
                    ch, j = nt // GE_EPI, nt % GE_EPI
                    if j == 0:
                        owide[0] = opool.tile([P, GE_EPI * HID], BF16,
                                              tag="ow", name="ow")
                    osl = owide[0][:, j * HID:(j + 1) * HID]
                    ve.tensor_scalar(
                        out=osl, in0=hb2[:], scalar1=0.0, scalar2=None,
                        op0=mybir.AluOpType.max)
                    ve.tensor_tensor(
                        out=osl, in0=osl, in1=t1[:],
                        op=mybir.AluOpType.add)
                    if j == GE_EPI - 1 or nt == cfg.nb - 1:
                        n0 = nt - j
                        # SWDGE cast bf16->f32 on the way out
                        nc.gpsimd.dma_start(
                            out=out_d[n0 * P:(nt + 1) * P, :].rearrange(
                                "(g p) f -> p g f", p=P),
                            in_=owide[0][:, :(j + 1) * HID].rearrange(
                                "p (g f) -> p g f", f=HID))

            # ---- layer pipeline
            alocal = dram.tile([cfg.shard, HID], BF16, tag="alocal",
                               name="alocal")
            for nt in range(cfg.nb):
                dense_block(0, nt, alocal)
            seg_lo = []
            s0 = 0
            for sg in cfg.segs:
                seg_lo.append(s0)
                s0 += sg
            for l in range(n_layers):
                p_ts = [drp.tile([r, HID], BF16, tag=f"p{si}", name=f"p{si}")
                        for si, r in enumerate(seg_rows)]
                rs_ts = body(l, alocal, p_ts)
                if l < n_layers - 1:
                    alocal = dram.tile([cfg.shard, HID], BF16, tag="alocal",
                                       name="alocal")
                # epilogues in chunks of GE blocks: one batched rs load each
                GE = GE_EPI
                for nt0 in range(0, cfg.nb, GE):
                    ng = min(GE, cfg.nb - nt0)
                    si = max(i for i in range(len(seg_lo))
                             if seg_lo[i] <= nt0)
                    assert nt0 + ng <= seg_lo[si] + cfg.segs[si]
                    rs_t, row0 = rs_ts[si], (nt0 - seg_lo[si]) * P
                    rsc = hpool.tile([P, GE * HID], BF16, tag="rsc",
                                     name="rsc")
                    nc.gpsimd.dma_start(
                        out=rsc[:, :ng * HID].rearrange("p (g f) -> p g f",
                                                        f=HID),
                        in_=rs_t[row0:row0 + ng * P, :].rearrange(
                            "(g p) f -> p g f", p=P))
                    for j in range(ng):
                        epilogue_block(l, nt0 + j,
                                       rsc[:, (j) * HID:(j + 1) * HID],
                                       alocal)

    nc.compile()
    return nc


# ----------------------------------------------------------------- execution

def _make_in_maps(cfg, prep, x, Ws, bs, prelu_a):
    xts = _pack_xts(cfg, np.asarray(x, np.float32), prep["gslot"])
    wcat = _pack_wcat(cfg, Ws)
    brep = np.zeros((P, 4 * cfg.hid), np.float32)
    for l, b in enumerate(bs):
        brep[:, l * cfg.hid:(l + 1) * cfg.hid] = b[None, :]
    arep = np.tile(np.asarray(prelu_a, np.float32)[None, :], (P, 1))
    iota = _iota_np()
    ident = np.eye(P, dtype=ml_dtypes.bfloat16)
    maps = []
    for c in range(cfg.ncores):
        maps.append({
            "gidx": prep["gidx"][c],
            "sdump": prep["sfull"][c],
            "dstc": prep["dstc"][c],
            "normc": prep["normc"][c],
            "iota": iota,
            "ident": ident,
            "wcat": wcat,
            "brep": brep,
            "arep": arep,
            "loopn": prep["loopn"][c],
            "xts": xts[c],
        })
    return maps


def _assemble_out(cfg, results, gslot):
    yperm = np.concatenate([results[c]["out"] for c in range(cfg.ncores)],
                           axis=0)
    return np.ascontiguousarray(yperm[gslot]).astype(np.float32)


def run(cfg, x, edge_index, edge_weight, W1, b1, W2, b2, W3, b3, W4, b4,
        prelu_a, return_nc=False):
    prep = _preprocess(cfg, edge_index, edge_weight)
    nc = _build(cfg, prep["T"])
    in_maps = _make_in_maps(cfg, prep, x,
                            [np.asarray(W1, np.float32), np.asarray(W2, np.float32),
                             np.asarray(W3, np.float32), np.asarray(W4, np.float32)],
                            [np.asarray(b1, np.float32), np.asarray(b2, np.float32),
                             np.asarray(b3, np.float32), np.asarray(b4, np.float32)],
                            np.asarray(prelu_a, np.float32))
    res = run_bass_kernel_spmd(nc, in_maps, core_ids=list(range(cfg.ncores)))
    y = _assemble_out(cfg, res.results, prep["gslot"])
    if return_nc:
        return y, nc, in_maps
    return y


def kernel(x, edge_index, edge_weight, W1, b1, W2, b2, W3, b3, W4, b4, prelu_a):
    return run(CFG, x, edge_index, edge_weight,
               W1, b1, W2, b2, W3, b3, W4, b4, prelu_a)
